# revision 2
# baseline (speedup 1.0000x reference)
"""Trainium2 Bass kernel for nn_MPDWConv (B=8, E=256, H=W=128), v2.

Sharding: data-parallel over batch — each of the 8 NeuronCores processes one
full image.

Per-core design (channel-major [c, h, w], fp16 datapath, f32 PSUM):
  * No guard padding anywhere: every conv tap is emitted on the clipped
    row/col range it is valid for (center tap first covers the full window,
    so PSUM accumulation / in-place chains stay exact at the borders).
  * x arrives via banded strided HBM->SBUF DMAs straight into [128,128,128]
    tiles; y leaves straight from PSUM (f32) after the pointwise GEMM.
  * b_pw is folded upstream: delta = w_pw^-1 @ b_pw is added to the biases
    of the xc-producing stages (chunk0 via S1-evac bias on ch0..63, branches
    via their evac/chain biases), so the PW needs no bias at all.
  * Depthwise work (stage-1 3x3, branch 3x3 dil3, branch 5x5 dil3) is
    distributed across all four engines, per window-pair (8 rows):
      'p': diagonal matmuls on PE accumulating in PSUM + Act-evac w/ bias
      'v': DVE tensor_scalar (4x mode) products + tensor_tensor (2x) adds
      'c': Act activation products (scale/bias APs) + DVE tensor_tensor adds
      'g': DVE first tap (w/ bias), then GpSimd scalar_tensor_tensor chain
  * Pointwise conv: dense fp16 GEMM on PE (2 K-chunks x 2 out-blocks per
    4-row window), PSUM -> HBM DMA issued by the sync engine.
"""

import os as _os

import numpy as np

B, E, H, W = 8, 256, 128, 128

# Tap offset tables, center (0,0) first so the first op of every scheme
# covers the full window.
def _mk_taps(offs):
    taps = [(dy, dx) for dy in offs for dx in offs]
    taps.remove((0, 0))
    taps.sort(key=lambda t: (t[0] > 0, t))
    return [(0, 0)] + taps

TAPS_S1 = _mk_taps((-1, 0, 1))            # stage-1 3x3, dilation 1
TAPS_B1 = _mk_taps((-3, 0, 3))            # branch 3x3, dilation 3
TAPS_B2 = _mk_taps((-6, -3, 0, 3, 6))     # branch 5x5, dilation 3

# Engine assignment (tunable via env for sweeps):
#  S1A: 32 slots, window-pair j in 0..15 x blk in 0..1 (index j*2+blk)
#  B1A/B2A: 16 slots each (per window-pair)
S1A = _os.environ.get("S1A", "pppppp" + "cv" * 13)
B1A = _os.environ.get("B1A", "z" * 14 + "cc")
# chunk0 passthrough copy engine per pair: g=GpSimd, v=DVE, a=Act
CPY = _os.environ.get("CPY", "g" * 16)
# of the br1 dy<=0 taps, how many go to PE in scheme z
Z_NPE = int(_os.environ.get("Z_NPE", "6"))
# of the 8 non-center adds in scheme c, how many go to Pool
C_NPOOL = int(_os.environ.get("C_NPOOL", "0"))
B2A = _os.environ.get("B2A", "p" * 14 + "yy")
# PW PSUM->SBUF evac engine per 4-row window (32 slots): a=Act, v=DVE, g=GpSimd
PWE = _os.environ.get("PWE", "a" * 28 + "avav")
LAG = int(_os.environ.get("LAG", "1"))

_CACHE = {}


def _split_excess_waits(nc, mybir):
    """Walrus legalization: TRN2 instructions encode at most ONE sync wait
    (two for EventSemaphore). Tile attaches multi-wait sync_info; move the
    excess onto same-engine NoOp prefixes."""
    n_created = 0
    for fn in nc.m.functions:
        for blk in fn.blocks:
            insts = list(blk.instructions)
            out = []
            changed = False
            for inst in insts:
                si = getattr(inst, "sync_info", None)
                cap = 2 if isinstance(inst, mybir.InstEventSemaphore) else 1
                if si is not None and si.on_wait is not None \
                        and len(si.on_wait) > cap:
                    waits = list(si.on_wait)
                    extra, keep = waits[:-cap], waits[-cap:]
                    for w in extra:
                        n_created += 1
                        nop = mybir.InstNoOp(
                            name=f"I-waitsplit-{n_created}",
                            engine=inst.engine)
                        nop.sync_info = mybir.SyncInfo(
                            on_wait=[w], on_update=[])
                        out.append(nop)
                    inst.sync_info = mybir.SyncInfo(
                        on_wait=keep, on_update=list(si.on_update))
                    changed = True
                out.append(inst)
            if changed:
                blk.instructions = out
    return n_created


def _clip(dy, dx, r0, hgt):
    """Valid local (row, col) ranges of a window [r0, r0+hgt) x [0, 128)
    for a tap reading (row+dy, col+dx). Returns None if empty."""
    rlo = max(0, -r0 - dy)
    rhi = min(hgt, 128 - r0 - dy)
    clo = max(0, -dx)
    chi = min(128, 128 - dx)
    if rhi <= rlo or chi <= clo:
        return None
    return rlo, rhi, clo, chi


def _build_nc():
    import concourse.bass as bass
    import concourse.mybir as mybir
    from concourse import tile

    f16 = mybir.dt.float16
    f32 = mybir.dt.float32
    mult, add = mybir.AluOpType.mult, mybir.AluOpType.add
    IDENT = mybir.ActivationFunctionType.Identity

    nc = bass.Bass(trn_type="TRN2")

    # ---- DRAM parameters ----
    xb = nc.dram_tensor("xb", [2, 128, H, W], f16, kind="ExternalInput")
    d0 = nc.dram_tensor("d0", [2, 128, 9 * 128], f16, kind="ExternalInput")
    d1 = nc.dram_tensor("d1", [128, 9 * 128], f16, kind="ExternalInput")
    d2 = nc.dram_tensor("d2", [128, 25 * 128], f16, kind="ExternalInput")
    wpw = nc.dram_tensor("wpw", [2, 128, 256], f16, kind="ExternalInput")
    k0s = nc.dram_tensor("k0s", [2, 128, 9], f32, kind="ExternalInput")
    k1s = nc.dram_tensor("k1s", [128, 9], f32, kind="ExternalInput")
    k2s = nc.dram_tensor("k2s", [128, 25], f32, kind="ExternalInput")
    be0 = nc.dram_tensor("be0", [128, 1], f32, kind="ExternalInput")
    be1 = nc.dram_tensor("be1", [128, 1], f32, kind="ExternalInput")
    bb1 = nc.dram_tensor("bb1", [128, 1], f32, kind="ExternalInput")
    bb2 = nc.dram_tensor("bb2", [128, 1], f32, kind="ExternalInput")
    bpw = nc.dram_tensor("bpw", [2, 128, 1], f32, kind="ExternalInput")
    y = nc.dram_tensor("y", [E, H, W], f16, kind="ExternalOutput")

    xb_ap, y_ap = xb.ap(), y.ap()

    with tile.TileContext(nc) as tc:
        with (
            tc.tile_pool(name="const", bufs=1) as cpool,
            tc.tile_pool(name="xin", bufs=1) as xpool,
            tc.tile_pool(name="x0", bufs=1) as x0pool,
            tc.tile_pool(name="xcg", bufs=4) as xcpool,
            tc.tile_pool(name="tmps", bufs=3) as tmpool,
            tc.tile_pool(name="ys", bufs=3) as yspool,
            tc.tile_pool(name="ps_s1", bufs=2, space="PSUM") as ps1pool,
            tc.tile_pool(name="ps_br", bufs=2, space="PSUM") as psbrpool,
            tc.tile_pool(name="ps_pw", bufs=2, space="PSUM") as pspwpool,
        ):
            # ---- constants into SBUF (issued on sync engine / HWDGE) ----
            def cdma(shape, dt_, tag, src_ap):
                t = cpool.tile(shape, dt_, tag=tag, name=tag)
                nc.sync.dma_start(out=t[:], in_=src_ap)
                return t

            d0t = [cdma([128, 9 * 128], f16, f"d0_{b}", d0.ap()[b])
                   for b in range(2)]
            d1t = cdma([128, 9 * 128], f16, "d1", d1.ap())
            d2t = cdma([128, 25 * 128], f16, "d2", d2.ap())
            wpwt = [cdma([128, 256], f16, f"wpw_{k}", wpw.ap()[k])
                    for k in range(2)]
            k0t = [cdma([128, 9], f32, f"k0_{b}", k0s.ap()[b])
                   for b in range(2)]
            k1t = cdma([128, 9], f32, "k1", k1s.ap())
            k2t = cdma([128, 25], f32, "k2", k2s.ap())
            be0t = cdma([128, 1], f32, "be0", be0.ap())
            be1t = cdma([128, 1], f32, "be1", be1.ap())
            bb1t = cdma([128, 1], f32, "bb1", bb1.ap())
            bb2t = cdma([128, 1], f32, "bb2", bb2.ap())
            bpwt = [cdma([128, 1], f32, f"bpw_{ob}", bpw.ap()[ob])
                    for ob in range(2)]

            # Pre-touch scalar tiles on their consumer engines so steady-state
            # ops don't each carry an extra DMA-lane sync wait.
            scrV = cpool.tile([128, 1], f32, tag="scrV")
            scrA = cpool.tile([128, 1], f32, tag="scrA")
            scrG = cpool.tile([128, 1], f32, tag="scrG")
            for t in (k0t[0], k0t[1], k1t, k2t, be0t, be1t, bb1t, bb2t,
                      bpwt[0], bpwt[1]):
                nc.vector.tensor_copy(scrV[:], t[:, 0:1])
            for t in (k0t[0], k0t[1], k1t, k2t, be0t, be1t, bb1t, bb2t,
                      bpwt[0], bpwt[1]):
                nc.scalar.copy(scrA[:], t[:, 0:1])
            for t in (k0t[0], k0t[1], k1t, k2t):
                nc.gpsimd.tensor_copy(scrG[:], t[:, 0:1])

            # ---- input tiles: banded strided DMA, no padding ----
            xt = [xpool.tile([128, 128, 128], f16, tag=f"x{b}",
                              name=f"x{b}") for b in range(2)]
            bands = [(0, 8), (8, 8)] + [(r, 16) for r in range(16, 128, 16)]
            for r, h in bands:
                for blk in (1, 0):
                    nc.sync.dma_start(
                        out=xt[blk][:, r:r + h, :],
                        in_=xb_ap[blk, :, r:r + h, :])

            # ---- x0 tiles ----
            x0t = [x0pool.tile([128, 128, 128], f16, tag=f"x0_{b}",
                                name=f"x0_{b}") for b in range(2)]

            # ---------- scheme emitters ----------
            # Each emits one depthwise group over window rows [r0, r0+hgt)
            # writing `out_ap(rlo, rhi, clo, chi)` slices of the destination
            # (partition range already applied by caller via tiles/slices).

            def grp_pe(taps, src, dmat, bias_ap, dst, plo, r0, hgt, dr0,
                       pspool, tag):
                """PE diag matmuls into PSUM (4-row sub-windows) + Act evac.
                src rows are absolute; dst rows start at dr0."""
                for sub in range(0, hgt, 4):
                    rr = r0 + sub
                    ems = []
                    for t, (dy, dx) in enumerate(taps):
                        c = _clip(dy, dx, rr, 4)
                        if c is None:
                            continue
                        ems.append((t, dy, dx, c))
                    ps = pspool.tile([128, 4, 128], f32, tag=tag, name=tag)
                    n = len(ems)
                    for i, (t, dy, dx, (rlo, rhi, clo, chi)) in enumerate(ems):
                        nc.tensor.matmul(
                            ps[:, rlo:rhi, clo:chi],
                            lhsT=dmat[:, t * 128:(t + 1) * 128],
                            rhs=src[:, rr + dy + rlo: rr + dy + rhi,
                                    dx + clo: dx + chi],
                            start=(i == 0), stop=(i == n - 1),
                            skip_group_check=True,
                        )
                    nc.scalar.activation(
                        out=dst[plo:128, dr0 + sub: dr0 + sub + 4, :],
                        in_=ps[plo:128], func=IDENT,
                        bias=bias_ap[plo:128], scale=1.0,
                    )

            def grp_v(taps, src, ktile, bias_ap, dst, plo, r0, hgt, dr0):
                """DVE: ts (4x) center tap w/ bias -> dst, then per tap
                ts product (4x) + tt add (2x), in place on dst."""
                assert taps[0] == (0, 0)
                nc.vector.tensor_scalar(
                    out=dst[plo:128, dr0:dr0 + hgt, :],
                    in0=src[plo:128, r0:r0 + hgt, :],
                    scalar1=ktile[plo:128, 0:1], scalar2=bias_ap[plo:128],
                    op0=mult, op1=add)
                for t, (dy, dx) in enumerate(taps[1:], start=1):
                    c = _clip(dy, dx, r0, hgt)
                    if c is None:
                        continue
                    rlo, rhi, clo, chi = c
                    tmp = tmpool.tile([128, hgt, 128], f16, tag="vtmp", name="vtmp", bufs=4)
                    nc.vector.tensor_scalar(
                        out=tmp[plo:128, rlo:rhi, clo:chi],
                        in0=src[plo:128, r0 + dy + rlo: r0 + dy + rhi,
                                dx + clo: dx + chi],
                        scalar1=ktile[plo:128, t:t + 1], scalar2=None,
                        op0=mult)
                    nc.vector.tensor_add(
                        dst[plo:128, dr0 + rlo: dr0 + rhi, clo:chi],
                        dst[plo:128, dr0 + rlo: dr0 + rhi, clo:chi],
                        tmp[plo:128, rlo:rhi, clo:chi])

            def grp_c(taps, src, ktile, bias_ap, dst, plo, r0, hgt, dr0):
                """Act products (scale AP, bias on center) + DVE tt adds."""
                assert taps[0] == (0, 0)
                nc.scalar.activation(
                    out=dst[plo:128, dr0:dr0 + hgt, :],
                    in_=src[plo:128, r0:r0 + hgt, :],
                    func=IDENT, bias=bias_ap[plo:128],
                    scale=ktile[plo:128, 0:1])
                for t, (dy, dx) in enumerate(taps[1:], start=1):
                    c = _clip(dy, dx, r0, hgt)
                    if c is None:
                        continue
                    rlo, rhi, clo, chi = c
                    tmp = tmpool.tile([128, hgt, 128], f16, tag="ctmp", name="ctmp", bufs=8)
                    nc.scalar.activation(
                        out=tmp[plo:128, rlo:rhi, clo:chi],
                        in_=src[plo:128, r0 + dy + rlo: r0 + dy + rhi,
                                dx + clo: dx + chi],
                        func=IDENT, bias=0.0,
                        scale=ktile[plo:128, t:t + 1])
                    adder = (nc.gpsimd if t > len(taps) - 1 - C_NPOOL
                             else nc.vector)
                    adder.tensor_add(
                        dst[plo:128, dr0 + rlo: dr0 + rhi, clo:chi],
                        dst[plo:128, dr0 + rlo: dr0 + rhi, clo:chi],
                        tmp[plo:128, rlo:rhi, clo:chi])

            def grp_g(taps, src, ktile, bias_ap, dst, plo, r0, hgt, dr0):
                """DVE ts center tap w/ bias -> dst, then GpSimd STT chain."""
                assert taps[0] == (0, 0)
                nc.vector.tensor_scalar(
                    out=dst[plo:128, dr0:dr0 + hgt, :],
                    in0=src[plo:128, r0:r0 + hgt, :],
                    scalar1=ktile[plo:128, 0:1], scalar2=bias_ap[plo:128],
                    op0=mult, op1=add)
                for t, (dy, dx) in enumerate(taps[1:], start=1):
                    c = _clip(dy, dx, r0, hgt)
                    if c is None:
                        continue
                    rlo, rhi, clo, chi = c
                    nc.gpsimd.scalar_tensor_tensor(
                        out=dst[plo:128, dr0 + rlo: dr0 + rhi, clo:chi],
                        in0=src[plo:128, r0 + dy + rlo: r0 + dy + rhi,
                                dx + clo: dx + chi],
                        scalar=ktile[plo:128, t:t + 1],
                        in1=dst[plo:128, dr0 + rlo: dr0 + rhi, clo:chi],
                        op0=mult, op1=add)

            def grp_y(taps, src, dmat, ktile, bias_ap, dst, plo, r0, hgt,
                      dr0, pspool, tag):
                n_pe = 1 + sum(1 for dy, dx in taps[1:] if dy <= 0)
                grp_pe(taps[:n_pe], src, dmat, bias_ap, dst, plo, r0, hgt,
                       dr0, pspool, tag)
                for t, (dy, dx) in enumerate(taps):
                    if t < n_pe:
                        continue
                    c = _clip(dy, dx, r0, hgt)
                    if c is None:
                        continue
                    rlo, rhi, clo, chi = c
                    tmp = tmpool.tile([128, hgt, 128], f16, tag="vtmp",
                                      name="vtmp", bufs=4)
                    nc.vector.tensor_scalar(
                        out=tmp[plo:128, rlo:rhi, clo:chi],
                        in0=src[plo:128, r0 + dy + rlo: r0 + dy + rhi,
                                dx + clo: dx + chi],
                        scalar1=ktile[plo:128, t:t + 1], scalar2=None,
                        op0=mult)
                    nc.vector.tensor_add(
                        dst[plo:128, dr0 + rlo: dr0 + rhi, clo:chi],
                        dst[plo:128, dr0 + rlo: dr0 + rhi, clo:chi],
                        tmp[plo:128, rlo:rhi, clo:chi])

            def emit_group(kind, taps, src, dmat, ktile, bias_ap, dst, plo,
                           r0, hgt, dr0, pspool, tag):
                if kind == "y":
                    grp_y(taps, src, dmat, ktile, bias_ap, dst, plo, r0,
                          hgt, dr0, pspool, tag)
                elif kind == "p":
                    grp_pe(taps, src, dmat, bias_ap, dst, plo, r0, hgt, dr0,
                           pspool, tag)
                elif kind == "v":
                    grp_v(taps, src, ktile, bias_ap, dst, plo, r0, hgt, dr0)
                elif kind == "c":
                    grp_c(taps, src, ktile, bias_ap, dst, plo, r0, hgt, dr0)
                elif kind == "g":
                    grp_g(taps, src, ktile, bias_ap, dst, plo, r0, hgt, dr0)
                else:
                    raise ValueError(kind)

            # ---------- branch + pointwise for one window-pair ----------
            xcg = {}

            def emit_branch_early(j):
                """Emitted right after S1(j): chunk0 copy + the br1 taps
                that need no pair-(j+1) rows (dy <= 0)."""
                r0 = j * 8
                xc0 = xcpool.tile([128, 8, 128], f16, tag="xc0g", name="xc0g")
                xc1 = xcpool.tile([128, 8, 128], f16, tag="xc1g", name="xc1g")
                xcg[j] = (xc0, xc1)
                cp = CPY[j]
                csrc = x0t[0][0:64, r0:r0 + 8, :]
                if cp == "g":
                    nc.gpsimd.tensor_copy(xc0[0:64, :, :], csrc)
                elif cp == "a":
                    nc.scalar.copy(xc0[0:64, :, :], csrc)
                else:
                    nc.vector.tensor_copy(xc0[0:64, :, :], csrc)
                if B1A[j] == "z":
                    grp_pe(TAPS_B1[:Z_NPE], x0t[0][:], d1t[:], bb1t[:],
                           xc0[:], 64, r0, 8, 0, psbrpool, "br1")

            def emit_branch_late_z(j, xc0):
                """dy>0 br1 taps: DVE products + Pool tensor_add onto xc0."""
                r0 = j * 8
                for t, (dy, dx) in enumerate(TAPS_B1):
                    if t < Z_NPE:
                        continue
                    c = _clip(dy, dx, r0, 8)
                    if c is None:
                        continue
                    rlo, rhi, clo, chi = c
                    tmp = tmpool.tile([128, 8, 128], f16, tag="ztmp",
                                      name="ztmp", bufs=4)
                    nc.vector.tensor_scalar(
                        out=tmp[64:128, rlo:rhi, clo:chi],
                        in0=x0t[0][64:128, r0 + dy + rlo: r0 + dy + rhi,
                                   dx + clo: dx + chi],
                        scalar1=k1t[64:128, t:t + 1], scalar2=None,
                        op0=mult)
                    nc.gpsimd.tensor_add(
                        xc0[64:128, rlo:rhi, clo:chi],
                        xc0[64:128, rlo:rhi, clo:chi],
                        tmp[64:128, rlo:rhi, clo:chi])

            def emit_branch_pw(j):
                r0 = j * 8
                xc0, xc1 = xcg.pop(j)
                # branch2 (5x5 dil3) on x0 blk1 -> xc1 (all 128 ch)
                emit_group(B2A[j], TAPS_B2, x0t[1][:], d2t[:], k2t, bb2t[:],
                           xc1[:], 0, r0, 8, 0, psbrpool, "br2")
                # branch1 (3x3 dil3) on x0 blk0 ch64..127 -> xc0[64:]
                if B1A[j] == "z":
                    emit_branch_late_z(j, xc0)
                else:
                    emit_group(B1A[j], TAPS_B1, x0t[0][:], d1t[:], k1t,
                               bb1t[:], xc0[:], 64, r0, 8, 0, psbrpool,
                               "br1")
                # pointwise GEMM per 4-row window
                for sub in range(2):
                    rr = r0 + sub * 4
                    ev = PWE[j * 2 + sub]
                    for ob in range(2):
                        pw = pspwpool.tile([128, 4, 128], f32, tag="pw", name="pw")
                        nc.tensor.matmul(
                            pw[:], lhsT=wpwt[0][:, ob * 128:(ob + 1) * 128],
                            rhs=xc0[:, sub * 4:sub * 4 + 4, :],
                            start=True, stop=False)
                        nc.tensor.matmul(
                            pw[:], lhsT=wpwt[1][:, ob * 128:(ob + 1) * 128],
                            rhs=xc1[:, sub * 4:sub * 4 + 4, :],
                            start=False, stop=True)
                        ys = yspool.tile([128, 4, 128], f16, tag=f"ys{ob}",
                                         name=f"ys{ob}")
                        if ev == "a":
                            nc.scalar.activation(
                                out=ys[:], in_=pw[:], func=IDENT,
                                bias=bpwt[ob][:], scale=1.0)
                        else:
                            nc.vector.tensor_scalar(
                                out=ys[:], in0=pw[:], scalar1=1.0,
                                scalar2=bpwt[ob][:], op0=mult, op1=add)
                        nc.sync.dma_start(
                            out=y_ap[ob * 128:(ob + 1) * 128, rr:rr + 4, :],
                            in_=ys[:])

            # ---------- main pipeline ----------
            for j in range(16):
                for blk in (1, 0):
                    kind = S1A[j * 2 + blk]
                    emit_group(kind, TAPS_S1, xt[blk][:], d0t[blk][:],
                               k0t[blk], be0t[:] if blk == 0 else be1t[:],
                               x0t[blk][:], 0, j * 8, 8, j * 8, ps1pool,
                               "s1")
                emit_branch_early(j)
                if j >= LAG:
                    emit_branch_pw(j - LAG)
            for j in range(16 - LAG, 16):
                emit_branch_pw(j)
    return nc


def _prep_aux(w0, b0, w1, b1, w2, b2, w_pw, b_pw, f16):

    d0 = np.zeros((2, 128, 9 * 128), dtype=f16)
    k0sv = np.zeros((2, 128, 9), np.float32)
    for blk in range(2):
        for t, (dy, dx) in enumerate(TAPS_S1):
            vals = w0[blk * 128:(blk + 1) * 128, 0, dy + 1, dx + 1]
            np.fill_diagonal(d0[blk, :, t * 128:(t + 1) * 128],
                             vals.astype(f16))
            k0sv[blk, :, t] = vals
    d1 = np.zeros((128, 9 * 128), dtype=f16)
    k1sv = np.zeros((128, 9), np.float32)
    for t, (dy, dx) in enumerate(TAPS_B1):
        vals = np.zeros(128, np.float32)
        vals[64:128] = w1[:, 0, dy // 3 + 1, dx // 3 + 1]
        np.fill_diagonal(d1[:, t * 128:(t + 1) * 128], vals.astype(f16))
        k1sv[:, t] = vals
    d2 = np.zeros((128, 25 * 128), dtype=f16)
    k2sv = np.zeros((128, 25), np.float32)
    for t, (dy, dx) in enumerate(TAPS_B2):
        v = w2[:, 0, dy // 3 + 2, dx // 3 + 2]
        vals = np.concatenate([v, v])
        np.fill_diagonal(d2[:, t * 128:(t + 1) * 128], vals.astype(f16))
        k2sv[:, t] = vals
    wpw = np.zeros((2, 128, 256), dtype=f16)
    for k in range(2):
        wpw[k] = np.ascontiguousarray(
            w_pw[:, k * 128:(k + 1) * 128].T).astype(f16)

    be0 = b0[0:128].copy()
    be1 = b0[128:256].copy()
    bb1 = np.concatenate([np.zeros(64, np.float32), b1])
    bb2 = np.concatenate([b2, b2])
    return dict(
        d0=d0, d1=d1, d2=d2, wpw=wpw, k0s=k0sv, k1s=k1sv, k2s=k2sv,
        be0=be0.reshape(128, 1).astype(np.float32),
        be1=be1.reshape(128, 1).astype(np.float32),
        bb1=bb1.reshape(128, 1).astype(np.float32),
        bb2=bb2.reshape(128, 1).astype(np.float32),
        bpw=b_pw.reshape(2, 128, 1).astype(np.float32),
    )


def kernel(x, w0, b0, w1, b1, w2, b2, w_pw, b_pw):
    import concourse.mybir as mybir
    from concourse.bass_utils import run_bass_kernel_spmd

    f16 = mybir.dt.np(mybir.dt.float16)

    if "nc" not in _CACHE:
        nc = _build_nc()
        _split_excess_waits(nc, mybir)
        _CACHE["nc"] = nc
    nc = _CACHE["nc"]

    x = np.asarray(x, np.float32)
    aux = _prep_aux(
        np.asarray(w0, np.float32), np.asarray(b0, np.float32),
        np.asarray(w1, np.float32), np.asarray(b1, np.float32),
        np.asarray(w2, np.float32), np.asarray(b2, np.float32),
        np.asarray(w_pw, np.float32), np.asarray(b_pw, np.float32),
        f16,
    )
    in_maps = [
        {"xb": np.ascontiguousarray(x[i].reshape(2, 128, H, W)).astype(f16),
         **aux}
        for i in range(B)
    ]
    res = run_bass_kernel_spmd(nc, in_maps, core_ids=list(range(B)))
    _CACHE["last_result"] = res
    return np.stack([res.results[i]["y"] for i in range(B)]).astype(np.float32)


# revision 3
# speedup vs baseline: 1.0030x; 1.0030x over previous
"""Trainium2 Bass kernel for nn_MPDWConv (B=8, E=256, H=W=128), v2.

Sharding: data-parallel over batch — each of the 8 NeuronCores processes one
full image.

Per-core design (channel-major [c, h, w], fp16 datapath, f32 PSUM):
  * No guard padding anywhere: every conv tap is emitted on the clipped
    row/col range it is valid for (center tap first covers the full window,
    so PSUM accumulation / in-place chains stay exact at the borders).
  * x arrives via banded strided HBM->SBUF DMAs straight into [128,128,128]
    tiles; y leaves straight from PSUM (f32) after the pointwise GEMM.
  * b_pw is folded upstream: delta = w_pw^-1 @ b_pw is added to the biases
    of the xc-producing stages (chunk0 via S1-evac bias on ch0..63, branches
    via their evac/chain biases), so the PW needs no bias at all.
  * Depthwise work (stage-1 3x3, branch 3x3 dil3, branch 5x5 dil3) is
    distributed across all four engines, per window-pair (8 rows):
      'p': diagonal matmuls on PE accumulating in PSUM + Act-evac w/ bias
      'v': DVE tensor_scalar (4x mode) products + tensor_tensor (2x) adds
      'c': Act activation products (scale/bias APs) + DVE tensor_tensor adds
      'g': DVE first tap (w/ bias), then GpSimd scalar_tensor_tensor chain
  * Pointwise conv: dense fp16 GEMM on PE (2 K-chunks x 2 out-blocks per
    4-row window), PSUM -> HBM DMA issued by the sync engine.
"""

import os as _os

import numpy as np

B, E, H, W = 8, 256, 128, 128

# Tap offset tables, center (0,0) first so the first op of every scheme
# covers the full window.
def _mk_taps(offs):
    taps = [(dy, dx) for dy in offs for dx in offs]
    taps.remove((0, 0))
    taps.sort(key=lambda t: (t[0] > 0, t))
    return [(0, 0)] + taps

TAPS_S1 = _mk_taps((-1, 0, 1))            # stage-1 3x3, dilation 1
TAPS_B1 = _mk_taps((-3, 0, 3))            # branch 3x3, dilation 3
TAPS_B2 = _mk_taps((-6, -3, 0, 3, 6))     # branch 5x5, dilation 3

# Engine assignment (tunable via env for sweeps):
#  S1A: 32 slots, window-pair j in 0..15 x blk in 0..1 (index j*2+blk)
#  B1A/B2A: 16 slots each (per window-pair)
S1A = _os.environ.get("S1A", "pppppp" + "cv" * 13)
B1A = _os.environ.get("B1A", "z" * 14 + "cc")
# chunk0 passthrough copy engine per pair: g=GpSimd, v=DVE, a=Act
CPY = _os.environ.get("CPY", "g" * 16)
# of the br1 dy<=0 taps, how many go to PE in scheme z
Z_NPE = int(_os.environ.get("Z_NPE", "6"))
# of the 8 non-center adds in scheme c, how many go to Pool
C_NPOOL = int(_os.environ.get("C_NPOOL", "0"))
B2A = _os.environ.get("B2A", "p" * 14 + "yy")
# PW PSUM->SBUF evac engine per 4-row window (32 slots): a=Act, v=DVE, g=GpSimd
PWE = _os.environ.get("PWE", "a" * 28 + "avav")
LAG = int(_os.environ.get("LAG", "1"))
S1R4 = int(_os.environ.get("S1R4", "0"))

_CACHE = {}


def _split_excess_waits(nc, mybir):
    """Walrus legalization: TRN2 instructions encode at most ONE sync wait
    (two for EventSemaphore). Tile attaches multi-wait sync_info; move the
    excess onto same-engine NoOp prefixes."""
    n_created = 0
    for fn in nc.m.functions:
        for blk in fn.blocks:
            insts = list(blk.instructions)
            out = []
            changed = False
            for inst in insts:
                si = getattr(inst, "sync_info", None)
                cap = 2 if isinstance(inst, mybir.InstEventSemaphore) else 1
                if si is not None and si.on_wait is not None \
                        and len(si.on_wait) > cap:
                    waits = list(si.on_wait)
                    extra, keep = waits[:-cap], waits[-cap:]
                    for w in extra:
                        n_created += 1
                        nop = mybir.InstNoOp(
                            name=f"I-waitsplit-{n_created}",
                            engine=inst.engine)
                        nop.sync_info = mybir.SyncInfo(
                            on_wait=[w], on_update=[])
                        out.append(nop)
                    inst.sync_info = mybir.SyncInfo(
                        on_wait=keep, on_update=list(si.on_update))
                    changed = True
                out.append(inst)
            if changed:
                blk.instructions = out
    return n_created


def _clip(dy, dx, r0, hgt):
    """Valid local (row, col) ranges of a window [r0, r0+hgt) x [0, 128)
    for a tap reading (row+dy, col+dx). Returns None if empty."""
    rlo = max(0, -r0 - dy)
    rhi = min(hgt, 128 - r0 - dy)
    clo = max(0, -dx)
    chi = min(128, 128 - dx)
    if rhi <= rlo or chi <= clo:
        return None
    return rlo, rhi, clo, chi


def _build_nc():
    import concourse.bass as bass
    import concourse.mybir as mybir
    from concourse import tile

    f16 = mybir.dt.float16
    f32 = mybir.dt.float32
    mult, add = mybir.AluOpType.mult, mybir.AluOpType.add
    IDENT = mybir.ActivationFunctionType.Identity

    nc = bass.Bass(trn_type="TRN2")

    # ---- DRAM parameters ----
    xb = nc.dram_tensor("xb", [2, 128, H, W], f16, kind="ExternalInput")
    d0 = nc.dram_tensor("d0", [2, 128, 9 * 128], f16, kind="ExternalInput")
    d1 = nc.dram_tensor("d1", [128, 9 * 128], f16, kind="ExternalInput")
    d2 = nc.dram_tensor("d2", [128, 25 * 128], f16, kind="ExternalInput")
    wpw = nc.dram_tensor("wpw", [2, 128, 256], f16, kind="ExternalInput")
    k0s = nc.dram_tensor("k0s", [2, 128, 9], f32, kind="ExternalInput")
    k1s = nc.dram_tensor("k1s", [128, 9], f32, kind="ExternalInput")
    k2s = nc.dram_tensor("k2s", [128, 25], f32, kind="ExternalInput")
    be0 = nc.dram_tensor("be0", [128, 1], f32, kind="ExternalInput")
    be1 = nc.dram_tensor("be1", [128, 1], f32, kind="ExternalInput")
    bb1 = nc.dram_tensor("bb1", [128, 1], f32, kind="ExternalInput")
    bb2 = nc.dram_tensor("bb2", [128, 1], f32, kind="ExternalInput")
    bpw = nc.dram_tensor("bpw", [2, 128, 1], f32, kind="ExternalInput")
    y = nc.dram_tensor("y", [E, H, W], f16, kind="ExternalOutput")

    xb_ap, y_ap = xb.ap(), y.ap()

    with tile.TileContext(nc) as tc:
        with (
            tc.tile_pool(name="const", bufs=1) as cpool,
            tc.tile_pool(name="xin", bufs=1) as xpool,
            tc.tile_pool(name="x0", bufs=1) as x0pool,
            tc.tile_pool(name="xcg", bufs=4) as xcpool,
            tc.tile_pool(name="tmps", bufs=3) as tmpool,
            tc.tile_pool(name="ys", bufs=3) as yspool,
            tc.tile_pool(name="ps_s1", bufs=1, space="PSUM") as ps1pool,
            tc.tile_pool(name="ps_br", bufs=2, space="PSUM") as psbrpool,
            tc.tile_pool(name="ps_pw", bufs=3, space="PSUM") as pspwpool,
        ):
            # ---- constants into SBUF (issued on sync engine / HWDGE) ----
            def cdma(shape, dt_, tag, src_ap):
                t = cpool.tile(shape, dt_, tag=tag, name=tag)
                nc.sync.dma_start(out=t[:], in_=src_ap)
                return t

            d0t = [cdma([128, 9 * 128], f16, f"d0_{b}", d0.ap()[b])
                   for b in range(2)]
            d1t = cdma([128, 9 * 128], f16, "d1", d1.ap())
            d2t = cdma([128, 25 * 128], f16, "d2", d2.ap())
            wpwt = [cdma([128, 256], f16, f"wpw_{k}", wpw.ap()[k])
                    for k in range(2)]
            k0t = [cdma([128, 9], f32, f"k0_{b}", k0s.ap()[b])
                   for b in range(2)]
            k1t = cdma([128, 9], f32, "k1", k1s.ap())
            k2t = cdma([128, 25], f32, "k2", k2s.ap())
            be0t = cdma([128, 1], f32, "be0", be0.ap())
            be1t = cdma([128, 1], f32, "be1", be1.ap())
            bb1t = cdma([128, 1], f32, "bb1", bb1.ap())
            bb2t = cdma([128, 1], f32, "bb2", bb2.ap())
            bpwt = [cdma([128, 1], f32, f"bpw_{ob}", bpw.ap()[ob])
                    for ob in range(2)]

            # Pre-touch scalar tiles on their consumer engines so steady-state
            # ops don't each carry an extra DMA-lane sync wait.
            scrV = cpool.tile([128, 1], f32, tag="scrV")
            scrA = cpool.tile([128, 1], f32, tag="scrA")
            scrG = cpool.tile([128, 1], f32, tag="scrG")
            for t in (k0t[0], k0t[1], k1t, k2t, be0t, be1t, bb1t, bb2t,
                      bpwt[0], bpwt[1]):
                nc.vector.tensor_copy(scrV[:], t[:, 0:1])
            for t in (k0t[0], k0t[1], k1t, k2t, be0t, be1t, bb1t, bb2t,
                      bpwt[0], bpwt[1]):
                nc.scalar.copy(scrA[:], t[:, 0:1])
            for t in (k0t[0], k0t[1], k1t, k2t):
                nc.gpsimd.tensor_copy(scrG[:], t[:, 0:1])

            # ---- input tiles: banded strided DMA, no padding ----
            xt = [xpool.tile([128, 128, 128], f16, tag=f"x{b}",
                              name=f"x{b}") for b in range(2)]
            bands = [(0, 8), (8, 8)] + [(r, 16) for r in range(16, 128, 16)]
            for r, h in bands:
                for blk in (1, 0):
                    nc.sync.dma_start(
                        out=xt[blk][:, r:r + h, :],
                        in_=xb_ap[blk, :, r:r + h, :])

            # ---- x0 tiles ----
            x0t = [x0pool.tile([128, 128, 128], f16, tag=f"x0_{b}",
                                name=f"x0_{b}") for b in range(2)]

            # ---------- scheme emitters ----------
            # Each emits one depthwise group over window rows [r0, r0+hgt)
            # writing `out_ap(rlo, rhi, clo, chi)` slices of the destination
            # (partition range already applied by caller via tiles/slices).

            def grp_pe(taps, src, dmat, bias_ap, dst, plo, r0, hgt, dr0,
                       pspool, tag):
                """PE diag matmuls into PSUM (4-row sub-windows) + Act evac.
                src rows are absolute; dst rows start at dr0."""
                for sub in range(0, hgt, 4):
                    rr = r0 + sub
                    ems = []
                    for t, (dy, dx) in enumerate(taps):
                        c = _clip(dy, dx, rr, 4)
                        if c is None:
                            continue
                        ems.append((t, dy, dx, c))
                    ps = pspool.tile([128, 4, 128], f32, tag=tag, name=tag)
                    n = len(ems)
                    for i, (t, dy, dx, (rlo, rhi, clo, chi)) in enumerate(ems):
                        nc.tensor.matmul(
                            ps[:, rlo:rhi, clo:chi],
                            lhsT=dmat[:, t * 128:(t + 1) * 128],
                            rhs=src[:, rr + dy + rlo: rr + dy + rhi,
                                    dx + clo: dx + chi],
                            start=(i == 0), stop=(i == n - 1),
                            skip_group_check=True,
                        )
                    nc.scalar.activation(
                        out=dst[plo:128, dr0 + sub: dr0 + sub + 4, :],
                        in_=ps[plo:128], func=IDENT,
                        bias=bias_ap[plo:128], scale=1.0,
                    )

            def grp_v(taps, src, ktile, bias_ap, dst, plo, r0, hgt, dr0):
                """DVE: ts (4x) center tap w/ bias -> dst, then per tap
                ts product (4x) + tt add (2x), in place on dst."""
                assert taps[0] == (0, 0)
                nc.vector.tensor_scalar(
                    out=dst[plo:128, dr0:dr0 + hgt, :],
                    in0=src[plo:128, r0:r0 + hgt, :],
                    scalar1=ktile[plo:128, 0:1], scalar2=bias_ap[plo:128],
                    op0=mult, op1=add)
                for t, (dy, dx) in enumerate(taps[1:], start=1):
                    c = _clip(dy, dx, r0, hgt)
                    if c is None:
                        continue
                    rlo, rhi, clo, chi = c
                    tmp = tmpool.tile([128, hgt, 128], f16, tag="vtmp", name="vtmp", bufs=6)
                    nc.vector.tensor_scalar(
                        out=tmp[plo:128, rlo:rhi, clo:chi],
                        in0=src[plo:128, r0 + dy + rlo: r0 + dy + rhi,
                                dx + clo: dx + chi],
                        scalar1=ktile[plo:128, t:t + 1], scalar2=None,
                        op0=mult)
                    nc.vector.tensor_add(
                        dst[plo:128, dr0 + rlo: dr0 + rhi, clo:chi],
                        dst[plo:128, dr0 + rlo: dr0 + rhi, clo:chi],
                        tmp[plo:128, rlo:rhi, clo:chi])

            def grp_c(taps, src, ktile, bias_ap, dst, plo, r0, hgt, dr0):
                """Act products (scale AP, bias on center) + DVE tt adds."""
                assert taps[0] == (0, 0)
                nc.scalar.activation(
                    out=dst[plo:128, dr0:dr0 + hgt, :],
                    in_=src[plo:128, r0:r0 + hgt, :],
                    func=IDENT, bias=bias_ap[plo:128],
                    scale=ktile[plo:128, 0:1])
                for t, (dy, dx) in enumerate(taps[1:], start=1):
                    c = _clip(dy, dx, r0, hgt)
                    if c is None:
                        continue
                    rlo, rhi, clo, chi = c
                    tmp = tmpool.tile([128, hgt, 128], f16, tag="ctmp", name="ctmp", bufs=8)
                    nc.scalar.activation(
                        out=tmp[plo:128, rlo:rhi, clo:chi],
                        in_=src[plo:128, r0 + dy + rlo: r0 + dy + rhi,
                                dx + clo: dx + chi],
                        func=IDENT, bias=0.0,
                        scale=ktile[plo:128, t:t + 1])
                    adder = (nc.gpsimd if t > len(taps) - 1 - C_NPOOL
                             else nc.vector)
                    adder.tensor_add(
                        dst[plo:128, dr0 + rlo: dr0 + rhi, clo:chi],
                        dst[plo:128, dr0 + rlo: dr0 + rhi, clo:chi],
                        tmp[plo:128, rlo:rhi, clo:chi])

            def grp_g(taps, src, ktile, bias_ap, dst, plo, r0, hgt, dr0):
                """DVE ts center tap w/ bias -> dst, then GpSimd STT chain."""
                assert taps[0] == (0, 0)
                nc.vector.tensor_scalar(
                    out=dst[plo:128, dr0:dr0 + hgt, :],
                    in0=src[plo:128, r0:r0 + hgt, :],
                    scalar1=ktile[plo:128, 0:1], scalar2=bias_ap[plo:128],
                    op0=mult, op1=add)
                for t, (dy, dx) in enumerate(taps[1:], start=1):
                    c = _clip(dy, dx, r0, hgt)
                    if c is None:
                        continue
                    rlo, rhi, clo, chi = c
                    nc.gpsimd.scalar_tensor_tensor(
                        out=dst[plo:128, dr0 + rlo: dr0 + rhi, clo:chi],
                        in0=src[plo:128, r0 + dy + rlo: r0 + dy + rhi,
                                dx + clo: dx + chi],
                        scalar=ktile[plo:128, t:t + 1],
                        in1=dst[plo:128, dr0 + rlo: dr0 + rhi, clo:chi],
                        op0=mult, op1=add)

            def grp_y(taps, src, dmat, ktile, bias_ap, dst, plo, r0, hgt,
                      dr0, pspool, tag):
                n_pe = 1 + sum(1 for dy, dx in taps[1:] if dy <= 0)
                grp_pe(taps[:n_pe], src, dmat, bias_ap, dst, plo, r0, hgt,
                       dr0, pspool, tag)
                for t, (dy, dx) in enumerate(taps):
                    if t < n_pe:
                        continue
                    c = _clip(dy, dx, r0, hgt)
                    if c is None:
                        continue
                    rlo, rhi, clo, chi = c
                    tmp = tmpool.tile([128, hgt, 128], f16, tag="vtmp",
                                      name="vtmp", bufs=6)
                    nc.vector.tensor_scalar(
                        out=tmp[plo:128, rlo:rhi, clo:chi],
                        in0=src[plo:128, r0 + dy + rlo: r0 + dy + rhi,
                                dx + clo: dx + chi],
                        scalar1=ktile[plo:128, t:t + 1], scalar2=None,
                        op0=mult)
                    nc.vector.tensor_add(
                        dst[plo:128, dr0 + rlo: dr0 + rhi, clo:chi],
                        dst[plo:128, dr0 + rlo: dr0 + rhi, clo:chi],
                        tmp[plo:128, rlo:rhi, clo:chi])

            def emit_group(kind, taps, src, dmat, ktile, bias_ap, dst, plo,
                           r0, hgt, dr0, pspool, tag):
                if kind == "y":
                    grp_y(taps, src, dmat, ktile, bias_ap, dst, plo, r0,
                          hgt, dr0, pspool, tag)
                elif kind == "p":
                    grp_pe(taps, src, dmat, bias_ap, dst, plo, r0, hgt, dr0,
                           pspool, tag)
                elif kind == "v":
                    grp_v(taps, src, ktile, bias_ap, dst, plo, r0, hgt, dr0)
                elif kind == "c":
                    grp_c(taps, src, ktile, bias_ap, dst, plo, r0, hgt, dr0)
                elif kind == "g":
                    grp_g(taps, src, ktile, bias_ap, dst, plo, r0, hgt, dr0)
                else:
                    raise ValueError(kind)

            # ---------- branch + pointwise for one window-pair ----------
            xcg = {}

            def emit_branch_early(j):
                """Emitted right after S1(j): chunk0 copy + the br1 taps
                that need no pair-(j+1) rows (dy <= 0)."""
                r0 = j * 8
                xc0 = xcpool.tile([128, 8, 128], f16, tag="xc0g", name="xc0g")
                xc1 = xcpool.tile([128, 8, 128], f16, tag="xc1g", name="xc1g")
                xcg[j] = (xc0, xc1)
                cp = CPY[j]
                csrc = x0t[0][0:64, r0:r0 + 8, :]
                if cp == "g":
                    nc.gpsimd.tensor_copy(xc0[0:64, :, :], csrc)
                elif cp == "a":
                    nc.scalar.copy(xc0[0:64, :, :], csrc)
                else:
                    nc.vector.tensor_copy(xc0[0:64, :, :], csrc)
                if B1A[j] == "z":
                    grp_pe(TAPS_B1[:Z_NPE], x0t[0][:], d1t[:], bb1t[:],
                           xc0[:], 64, r0, 8, 0, psbrpool, "br1")

            def emit_branch_late_z(j, xc0):
                """dy>0 br1 taps: DVE products + Pool tensor_add onto xc0."""
                r0 = j * 8
                for t, (dy, dx) in enumerate(TAPS_B1):
                    if t < Z_NPE:
                        continue
                    c = _clip(dy, dx, r0, 8)
                    if c is None:
                        continue
                    rlo, rhi, clo, chi = c
                    tmp = tmpool.tile([128, 8, 128], f16, tag="ztmp",
                                      name="ztmp", bufs=6)
                    nc.vector.tensor_scalar(
                        out=tmp[64:128, rlo:rhi, clo:chi],
                        in0=x0t[0][64:128, r0 + dy + rlo: r0 + dy + rhi,
                                   dx + clo: dx + chi],
                        scalar1=k1t[64:128, t:t + 1], scalar2=None,
                        op0=mult)
                    nc.gpsimd.tensor_add(
                        xc0[64:128, rlo:rhi, clo:chi],
                        xc0[64:128, rlo:rhi, clo:chi],
                        tmp[64:128, rlo:rhi, clo:chi])

            def emit_branch_pw(j):
                r0 = j * 8
                xc0, xc1 = xcg.pop(j)
                # branch2 (5x5 dil3) on x0 blk1 -> xc1 (all 128 ch)
                emit_group(B2A[j], TAPS_B2, x0t[1][:], d2t[:], k2t, bb2t[:],
                           xc1[:], 0, r0, 8, 0, psbrpool, "br2")
                # branch1 (3x3 dil3) on x0 blk0 ch64..127 -> xc0[64:]
                if B1A[j] == "z":
                    emit_branch_late_z(j, xc0)
                else:
                    emit_group(B1A[j], TAPS_B1, x0t[0][:], d1t[:], k1t,
                               bb1t[:], xc0[:], 64, r0, 8, 0, psbrpool,
                               "br1")
                # pointwise GEMM per 4-row window
                for sub in range(2):
                    rr = r0 + sub * 4
                    ev = PWE[j * 2 + sub]
                    for ob in range(2):
                        pw = pspwpool.tile([128, 4, 128], f32, tag="pw", name="pw")
                        nc.tensor.matmul(
                            pw[:], lhsT=wpwt[0][:, ob * 128:(ob + 1) * 128],
                            rhs=xc0[:, sub * 4:sub * 4 + 4, :],
                            start=True, stop=False)
                        nc.tensor.matmul(
                            pw[:], lhsT=wpwt[1][:, ob * 128:(ob + 1) * 128],
                            rhs=xc1[:, sub * 4:sub * 4 + 4, :],
                            start=False, stop=True)
                        ys = yspool.tile([128, 4, 128], f16, tag=f"ys{ob}",
                                         name=f"ys{ob}")
                        if ev == "a":
                            nc.scalar.activation(
                                out=ys[:], in_=pw[:], func=IDENT,
                                bias=bpwt[ob][:], scale=1.0)
                        else:
                            nc.vector.tensor_scalar(
                                out=ys[:], in0=pw[:], scalar1=1.0,
                                scalar2=bpwt[ob][:], op0=mult, op1=add)
                        nc.sync.dma_start(
                            out=y_ap[ob * 128:(ob + 1) * 128, rr:rr + 4, :],
                            in_=ys[:])

            # ---------- main pipeline ----------
            def emit_s1(j, hgt):
                for blk in (1, 0):
                    kind = S1A[j * 2 + blk]
                    emit_group(kind, TAPS_S1, xt[blk][:], d0t[blk][:],
                               k0t[blk], be0t[:] if blk == 0 else be1t[:],
                               x0t[blk][:], 0, j * 8, hgt, j * 8, ps1pool,
                               "s1")

            if S1R4:
                # warmup pairs 0,1 (PE), then 16-row double-pair chains
                for j in (0, 1):
                    emit_s1(j, 8)
                    emit_branch_early(j)
                for d in range(1, 8):
                    emit_s1(2 * d, 16)
                    emit_branch_early(2 * d)
                    emit_branch_early(2 * d + 1)
                    emit_branch_pw(2 * d - 2)
                    emit_branch_pw(2 * d - 1)
                emit_branch_pw(14)
                emit_branch_pw(15)
            else:
                for j in range(16):
                    emit_s1(j, 8)
                    emit_branch_early(j)
                    if j >= LAG:
                        emit_branch_pw(j - LAG)
                for j in range(16 - LAG, 16):
                    emit_branch_pw(j)
    return nc


def _prep_aux(w0, b0, w1, b1, w2, b2, w_pw, b_pw, f16):

    d0 = np.zeros((2, 128, 9 * 128), dtype=f16)
    k0sv = np.zeros((2, 128, 9), np.float32)
    for blk in range(2):
        for t, (dy, dx) in enumerate(TAPS_S1):
            vals = w0[blk * 128:(blk + 1) * 128, 0, dy + 1, dx + 1]
            np.fill_diagonal(d0[blk, :, t * 128:(t + 1) * 128],
                             vals.astype(f16))
            k0sv[blk, :, t] = vals
    d1 = np.zeros((128, 9 * 128), dtype=f16)
    k1sv = np.zeros((128, 9), np.float32)
    for t, (dy, dx) in enumerate(TAPS_B1):
        vals = np.zeros(128, np.float32)
        vals[64:128] = w1[:, 0, dy // 3 + 1, dx // 3 + 1]
        np.fill_diagonal(d1[:, t * 128:(t + 1) * 128], vals.astype(f16))
        k1sv[:, t] = vals
    d2 = np.zeros((128, 25 * 128), dtype=f16)
    k2sv = np.zeros((128, 25), np.float32)
    for t, (dy, dx) in enumerate(TAPS_B2):
        v = w2[:, 0, dy // 3 + 2, dx // 3 + 2]
        vals = np.concatenate([v, v])
        np.fill_diagonal(d2[:, t * 128:(t + 1) * 128], vals.astype(f16))
        k2sv[:, t] = vals
    wpw = np.zeros((2, 128, 256), dtype=f16)
    for k in range(2):
        wpw[k] = np.ascontiguousarray(
            w_pw[:, k * 128:(k + 1) * 128].T).astype(f16)

    be0 = b0[0:128].copy()
    be1 = b0[128:256].copy()
    bb1 = np.concatenate([np.zeros(64, np.float32), b1])
    bb2 = np.concatenate([b2, b2])
    return dict(
        d0=d0, d1=d1, d2=d2, wpw=wpw, k0s=k0sv, k1s=k1sv, k2s=k2sv,
        be0=be0.reshape(128, 1).astype(np.float32),
        be1=be1.reshape(128, 1).astype(np.float32),
        bb1=bb1.reshape(128, 1).astype(np.float32),
        bb2=bb2.reshape(128, 1).astype(np.float32),
        bpw=b_pw.reshape(2, 128, 1).astype(np.float32),
    )


def kernel(x, w0, b0, w1, b1, w2, b2, w_pw, b_pw):
    import concourse.mybir as mybir
    from concourse.bass_utils import run_bass_kernel_spmd

    f16 = mybir.dt.np(mybir.dt.float16)

    if "nc" not in _CACHE:
        nc = _build_nc()
        _split_excess_waits(nc, mybir)
        _CACHE["nc"] = nc
    nc = _CACHE["nc"]

    x = np.asarray(x, np.float32)
    aux = _prep_aux(
        np.asarray(w0, np.float32), np.asarray(b0, np.float32),
        np.asarray(w1, np.float32), np.asarray(b1, np.float32),
        np.asarray(w2, np.float32), np.asarray(b2, np.float32),
        np.asarray(w_pw, np.float32), np.asarray(b_pw, np.float32),
        f16,
    )
    in_maps = [
        {"xb": np.ascontiguousarray(x[i].reshape(2, 128, H, W)).astype(f16),
         **aux}
        for i in range(B)
    ]
    res = run_bass_kernel_spmd(nc, in_maps, core_ids=list(range(B)))
    _CACHE["last_result"] = res
    return np.stack([res.results[i]["y"] for i in range(B)]).astype(np.float32)


# revision 4
# speedup vs baseline: 1.0333x; 1.0302x over previous
"""Trainium2 Bass kernel for nn_MPDWConv (B=8, E=256, H=W=128), v2.

Sharding: data-parallel over batch — each of the 8 NeuronCores processes one
full image.

Per-core design (channel-major [c, h, w], fp16 datapath, f32 PSUM):
  * No guard padding anywhere: every conv tap is emitted on the clipped
    row/col range it is valid for (center tap first covers the full window,
    so PSUM accumulation / in-place chains stay exact at the borders).
  * x arrives via banded strided HBM->SBUF DMAs straight into [128,128,128]
    tiles; y leaves straight from PSUM (f32) after the pointwise GEMM.
  * b_pw is folded upstream: delta = w_pw^-1 @ b_pw is added to the biases
    of the xc-producing stages (chunk0 via S1-evac bias on ch0..63, branches
    via their evac/chain biases), so the PW needs no bias at all.
  * Depthwise work (stage-1 3x3, branch 3x3 dil3, branch 5x5 dil3) is
    distributed across all four engines, per window-pair (8 rows):
      'p': diagonal matmuls on PE accumulating in PSUM + Act-evac w/ bias
      'v': DVE tensor_scalar (4x mode) products + tensor_tensor (2x) adds
      'c': Act activation products (scale/bias APs) + DVE tensor_tensor adds
      'g': DVE first tap (w/ bias), then GpSimd scalar_tensor_tensor chain
  * Pointwise conv: dense fp16 GEMM on PE (2 K-chunks x 2 out-blocks per
    4-row window), PSUM -> HBM DMA issued by the sync engine.
"""

import os as _os

import numpy as np

B, E, H, W = 8, 256, 128, 128

# Tap offset tables, center (0,0) first so the first op of every scheme
# covers the full window.
def _mk_taps(offs):
    taps = [(dy, dx) for dy in offs for dx in offs]
    taps.remove((0, 0))
    taps.sort(key=lambda t: (t[0] > 0, t))
    return [(0, 0)] + taps

TAPS_S1 = _mk_taps((-1, 0, 1))            # stage-1 3x3, dilation 1
TAPS_B1 = _mk_taps((-3, 0, 3))            # branch 3x3, dilation 3
TAPS_B2 = _mk_taps((-6, -3, 0, 3, 6))     # branch 5x5, dilation 3

# Engine assignment (tunable via env for sweeps):
#  S1A: 32 slots, window-pair j in 0..15 x blk in 0..1 (index j*2+blk)
#  B1A/B2A: 16 slots each (per window-pair)
S1A = _os.environ.get("S1A", "ppppvc" + "cv" * 13)
B1A = _os.environ.get("B1A", "z" * 14 + "cc")
# chunk0 passthrough copy engine per pair: g=GpSimd, v=DVE, a=Act
CPY = _os.environ.get("CPY", "g" * 16)
# of the br1 dy<=0 taps, how many go to PE in scheme z
Z_NPE = int(_os.environ.get("Z_NPE", "6"))
# of the 8 non-center adds in scheme c, how many go to Pool
C_NPOOL = int(_os.environ.get("C_NPOOL", "0"))
B2A = _os.environ.get("B2A", "p" * 14 + "yy")
# PW PSUM->SBUF evac engine per 4-row window (32 slots): a=Act, v=DVE, g=GpSimd
PWE = _os.environ.get("PWE", "a" * 28 + "avav")
LAG = int(_os.environ.get("LAG", "1"))
S1R4 = int(_os.environ.get("S1R4", "0"))

_CACHE = {}


def _split_excess_waits(nc, mybir):
    """Walrus legalization: TRN2 instructions encode at most ONE sync wait
    (two for EventSemaphore). Tile attaches multi-wait sync_info; move the
    excess onto same-engine NoOp prefixes."""
    n_created = 0
    for fn in nc.m.functions:
        for blk in fn.blocks:
            insts = list(blk.instructions)
            out = []
            changed = False
            for inst in insts:
                si = getattr(inst, "sync_info", None)
                cap = 2 if isinstance(inst, mybir.InstEventSemaphore) else 1
                if si is not None and si.on_wait is not None \
                        and len(si.on_wait) > cap:
                    waits = list(si.on_wait)
                    extra, keep = waits[:-cap], waits[-cap:]
                    for w in extra:
                        n_created += 1
                        nop = mybir.InstNoOp(
                            name=f"I-waitsplit-{n_created}",
                            engine=inst.engine)
                        nop.sync_info = mybir.SyncInfo(
                            on_wait=[w], on_update=[])
                        out.append(nop)
                    inst.sync_info = mybir.SyncInfo(
                        on_wait=keep, on_update=list(si.on_update))
                    changed = True
                out.append(inst)
            if changed:
                blk.instructions = out
    return n_created


def _clip(dy, dx, r0, hgt):
    """Valid local (row, col) ranges of a window [r0, r0+hgt) x [0, 128)
    for a tap reading (row+dy, col+dx). Returns None if empty."""
    rlo = max(0, -r0 - dy)
    rhi = min(hgt, 128 - r0 - dy)
    clo = max(0, -dx)
    chi = min(128, 128 - dx)
    if rhi <= rlo or chi <= clo:
        return None
    return rlo, rhi, clo, chi


def _build_nc():
    import concourse.bass as bass
    import concourse.mybir as mybir
    from concourse import tile

    f16 = mybir.dt.float16
    f32 = mybir.dt.float32
    mult, add = mybir.AluOpType.mult, mybir.AluOpType.add
    IDENT = mybir.ActivationFunctionType.Identity

    nc = bass.Bass(trn_type="TRN2")

    # ---- DRAM parameters ----
    xb = nc.dram_tensor("xb", [2, 128, H, W], f16, kind="ExternalInput")
    d0 = nc.dram_tensor("d0", [2, 128, 9 * 128], f16, kind="ExternalInput")
    d1 = nc.dram_tensor("d1", [128, 9 * 128], f16, kind="ExternalInput")
    d2 = nc.dram_tensor("d2", [128, 25 * 128], f16, kind="ExternalInput")
    wpw = nc.dram_tensor("wpw", [2, 128, 256], f16, kind="ExternalInput")
    k0s = nc.dram_tensor("k0s", [2, 128, 9], f32, kind="ExternalInput")
    k1s = nc.dram_tensor("k1s", [128, 9], f32, kind="ExternalInput")
    k2s = nc.dram_tensor("k2s", [128, 25], f32, kind="ExternalInput")
    be0 = nc.dram_tensor("be0", [128, 1], f32, kind="ExternalInput")
    be1 = nc.dram_tensor("be1", [128, 1], f32, kind="ExternalInput")
    bb1 = nc.dram_tensor("bb1", [128, 1], f32, kind="ExternalInput")
    bb2 = nc.dram_tensor("bb2", [128, 1], f32, kind="ExternalInput")
    bpw = nc.dram_tensor("bpw", [2, 128, 1], f32, kind="ExternalInput")
    y = nc.dram_tensor("y", [E, H, W], f16, kind="ExternalOutput")

    xb_ap, y_ap = xb.ap(), y.ap()

    with tile.TileContext(nc) as tc:
        with (
            tc.tile_pool(name="const", bufs=1) as cpool,
            tc.tile_pool(name="xin", bufs=1) as xpool,
            tc.tile_pool(name="x0", bufs=1) as x0pool,
            tc.tile_pool(name="xcg", bufs=4) as xcpool,
            tc.tile_pool(name="tmps", bufs=3) as tmpool,
            tc.tile_pool(name="ys", bufs=3) as yspool,
            tc.tile_pool(name="ps_s1", bufs=1, space="PSUM") as ps1pool,
            tc.tile_pool(name="ps_br", bufs=2, space="PSUM") as psbrpool,
            tc.tile_pool(name="ps_pw", bufs=3, space="PSUM") as pspwpool,
        ):
            # ---- constants into SBUF (issued on sync engine / HWDGE) ----
            def cdma(shape, dt_, tag, src_ap):
                t = cpool.tile(shape, dt_, tag=tag, name=tag)
                nc.sync.dma_start(out=t[:], in_=src_ap)
                return t

            d0t = [cdma([128, 9 * 128], f16, f"d0_{b}", d0.ap()[b])
                   for b in range(2)]
            d1t = cdma([128, 9 * 128], f16, "d1", d1.ap())
            d2t = cdma([128, 25 * 128], f16, "d2", d2.ap())
            wpwt = [cdma([128, 256], f16, f"wpw_{k}", wpw.ap()[k])
                    for k in range(2)]
            k0t = [cdma([128, 9], f32, f"k0_{b}", k0s.ap()[b])
                   for b in range(2)]
            k1t = cdma([128, 9], f32, "k1", k1s.ap())
            k2t = cdma([128, 25], f32, "k2", k2s.ap())
            be0t = cdma([128, 1], f32, "be0", be0.ap())
            be1t = cdma([128, 1], f32, "be1", be1.ap())
            bb1t = cdma([128, 1], f32, "bb1", bb1.ap())
            bb2t = cdma([128, 1], f32, "bb2", bb2.ap())
            bpwt = [cdma([128, 1], f32, f"bpw_{ob}", bpw.ap()[ob])
                    for ob in range(2)]

            # Pre-touch scalar tiles on their consumer engines so steady-state
            # ops don't each carry an extra DMA-lane sync wait.
            scrV = cpool.tile([128, 1], f32, tag="scrV")
            scrA = cpool.tile([128, 1], f32, tag="scrA")
            scrG = cpool.tile([128, 1], f32, tag="scrG")
            for t in (k0t[0], k0t[1], k1t, k2t, be0t, be1t, bb1t, bb2t,
                      bpwt[0], bpwt[1]):
                nc.vector.tensor_copy(scrV[:], t[:, 0:1])
            for t in (k0t[0], k0t[1], k1t, k2t, be0t, be1t, bb1t, bb2t,
                      bpwt[0], bpwt[1]):
                nc.scalar.copy(scrA[:], t[:, 0:1])
            for t in (k0t[0], k0t[1], k1t, k2t):
                nc.gpsimd.tensor_copy(scrG[:], t[:, 0:1])

            # ---- input tiles: banded strided DMA, no padding ----
            xt = [xpool.tile([128, 128, 128], f16, tag=f"x{b}",
                              name=f"x{b}") for b in range(2)]
            bands = [(0, 8), (8, 8)] + [(r, 16) for r in range(16, 128, 16)]
            for r, h in bands:
                for blk in (1, 0):
                    nc.sync.dma_start(
                        out=xt[blk][:, r:r + h, :],
                        in_=xb_ap[blk, :, r:r + h, :])

            # ---- x0 tiles ----
            x0t = [x0pool.tile([128, 128, 128], f16, tag=f"x0_{b}",
                                name=f"x0_{b}") for b in range(2)]

            # ---------- scheme emitters ----------
            # Each emits one depthwise group over window rows [r0, r0+hgt)
            # writing `out_ap(rlo, rhi, clo, chi)` slices of the destination
            # (partition range already applied by caller via tiles/slices).

            def grp_pe(taps, src, dmat, bias_ap, dst, plo, r0, hgt, dr0,
                       pspool, tag):
                """PE diag matmuls into PSUM (4-row sub-windows) + Act evac.
                src rows are absolute; dst rows start at dr0."""
                for sub in range(0, hgt, 4):
                    rr = r0 + sub
                    ems = []
                    for t, (dy, dx) in enumerate(taps):
                        c = _clip(dy, dx, rr, 4)
                        if c is None:
                            continue
                        ems.append((t, dy, dx, c))
                    ps = pspool.tile([128, 4, 128], f32, tag=tag, name=tag)
                    n = len(ems)
                    for i, (t, dy, dx, (rlo, rhi, clo, chi)) in enumerate(ems):
                        nc.tensor.matmul(
                            ps[:, rlo:rhi, clo:chi],
                            lhsT=dmat[:, t * 128:(t + 1) * 128],
                            rhs=src[:, rr + dy + rlo: rr + dy + rhi,
                                    dx + clo: dx + chi],
                            start=(i == 0), stop=(i == n - 1),
                            skip_group_check=True,
                        )
                    nc.scalar.activation(
                        out=dst[plo:128, dr0 + sub: dr0 + sub + 4, :],
                        in_=ps[plo:128], func=IDENT,
                        bias=bias_ap[plo:128], scale=1.0,
                    )

            def grp_v(taps, src, ktile, bias_ap, dst, plo, r0, hgt, dr0):
                """DVE: ts (4x) center tap w/ bias -> dst, then per tap
                ts product (4x) + tt add (2x), in place on dst."""
                assert taps[0] == (0, 0)
                nc.vector.tensor_scalar(
                    out=dst[plo:128, dr0:dr0 + hgt, :],
                    in0=src[plo:128, r0:r0 + hgt, :],
                    scalar1=ktile[plo:128, 0:1], scalar2=bias_ap[plo:128],
                    op0=mult, op1=add)
                for t, (dy, dx) in enumerate(taps[1:], start=1):
                    c = _clip(dy, dx, r0, hgt)
                    if c is None:
                        continue
                    rlo, rhi, clo, chi = c
                    tmp = tmpool.tile([128, hgt, 128], f16, tag="vtmp", name="vtmp", bufs=6)
                    nc.vector.tensor_scalar(
                        out=tmp[plo:128, rlo:rhi, clo:chi],
                        in0=src[plo:128, r0 + dy + rlo: r0 + dy + rhi,
                                dx + clo: dx + chi],
                        scalar1=ktile[plo:128, t:t + 1], scalar2=None,
                        op0=mult)
                    nc.vector.tensor_add(
                        dst[plo:128, dr0 + rlo: dr0 + rhi, clo:chi],
                        dst[plo:128, dr0 + rlo: dr0 + rhi, clo:chi],
                        tmp[plo:128, rlo:rhi, clo:chi])

            def grp_c(taps, src, ktile, bias_ap, dst, plo, r0, hgt, dr0):
                """Act products (scale AP, bias on center) + DVE tt adds."""
                assert taps[0] == (0, 0)
                nc.scalar.activation(
                    out=dst[plo:128, dr0:dr0 + hgt, :],
                    in_=src[plo:128, r0:r0 + hgt, :],
                    func=IDENT, bias=bias_ap[plo:128],
                    scale=ktile[plo:128, 0:1])
                for t, (dy, dx) in enumerate(taps[1:], start=1):
                    c = _clip(dy, dx, r0, hgt)
                    if c is None:
                        continue
                    rlo, rhi, clo, chi = c
                    tmp = tmpool.tile([128, hgt, 128], f16, tag="ctmp", name="ctmp", bufs=8)
                    nc.scalar.activation(
                        out=tmp[plo:128, rlo:rhi, clo:chi],
                        in_=src[plo:128, r0 + dy + rlo: r0 + dy + rhi,
                                dx + clo: dx + chi],
                        func=IDENT, bias=0.0,
                        scale=ktile[plo:128, t:t + 1])
                    adder = (nc.gpsimd if t > len(taps) - 1 - C_NPOOL
                             else nc.vector)
                    adder.tensor_add(
                        dst[plo:128, dr0 + rlo: dr0 + rhi, clo:chi],
                        dst[plo:128, dr0 + rlo: dr0 + rhi, clo:chi],
                        tmp[plo:128, rlo:rhi, clo:chi])

            def grp_g(taps, src, ktile, bias_ap, dst, plo, r0, hgt, dr0):
                """DVE ts center tap w/ bias -> dst, then GpSimd STT chain."""
                assert taps[0] == (0, 0)
                nc.vector.tensor_scalar(
                    out=dst[plo:128, dr0:dr0 + hgt, :],
                    in0=src[plo:128, r0:r0 + hgt, :],
                    scalar1=ktile[plo:128, 0:1], scalar2=bias_ap[plo:128],
                    op0=mult, op1=add)
                for t, (dy, dx) in enumerate(taps[1:], start=1):
                    c = _clip(dy, dx, r0, hgt)
                    if c is None:
                        continue
                    rlo, rhi, clo, chi = c
                    nc.gpsimd.scalar_tensor_tensor(
                        out=dst[plo:128, dr0 + rlo: dr0 + rhi, clo:chi],
                        in0=src[plo:128, r0 + dy + rlo: r0 + dy + rhi,
                                dx + clo: dx + chi],
                        scalar=ktile[plo:128, t:t + 1],
                        in1=dst[plo:128, dr0 + rlo: dr0 + rhi, clo:chi],
                        op0=mult, op1=add)

            def grp_y(taps, src, dmat, ktile, bias_ap, dst, plo, r0, hgt,
                      dr0, pspool, tag):
                n_pe = 1 + sum(1 for dy, dx in taps[1:] if dy <= 0)
                grp_pe(taps[:n_pe], src, dmat, bias_ap, dst, plo, r0, hgt,
                       dr0, pspool, tag)
                for t, (dy, dx) in enumerate(taps):
                    if t < n_pe:
                        continue
                    c = _clip(dy, dx, r0, hgt)
                    if c is None:
                        continue
                    rlo, rhi, clo, chi = c
                    tmp = tmpool.tile([128, hgt, 128], f16, tag="vtmp",
                                      name="vtmp", bufs=6)
                    nc.vector.tensor_scalar(
                        out=tmp[plo:128, rlo:rhi, clo:chi],
                        in0=src[plo:128, r0 + dy + rlo: r0 + dy + rhi,
                                dx + clo: dx + chi],
                        scalar1=ktile[plo:128, t:t + 1], scalar2=None,
                        op0=mult)
                    nc.vector.tensor_add(
                        dst[plo:128, dr0 + rlo: dr0 + rhi, clo:chi],
                        dst[plo:128, dr0 + rlo: dr0 + rhi, clo:chi],
                        tmp[plo:128, rlo:rhi, clo:chi])

            def emit_group(kind, taps, src, dmat, ktile, bias_ap, dst, plo,
                           r0, hgt, dr0, pspool, tag):
                if kind == "y":
                    grp_y(taps, src, dmat, ktile, bias_ap, dst, plo, r0,
                          hgt, dr0, pspool, tag)
                elif kind == "p":
                    grp_pe(taps, src, dmat, bias_ap, dst, plo, r0, hgt, dr0,
                           pspool, tag)
                elif kind == "v":
                    grp_v(taps, src, ktile, bias_ap, dst, plo, r0, hgt, dr0)
                elif kind == "c":
                    grp_c(taps, src, ktile, bias_ap, dst, plo, r0, hgt, dr0)
                elif kind == "g":
                    grp_g(taps, src, ktile, bias_ap, dst, plo, r0, hgt, dr0)
                else:
                    raise ValueError(kind)

            # ---------- branch + pointwise for one window-pair ----------
            xcg = {}

            def emit_branch_early(j):
                """Emitted right after S1(j): chunk0 copy + the br1 taps
                that need no pair-(j+1) rows (dy <= 0)."""
                r0 = j * 8
                xc0 = xcpool.tile([128, 8, 128], f16, tag="xc0g", name="xc0g")
                xc1 = xcpool.tile([128, 8, 128], f16, tag="xc1g", name="xc1g")
                xcg[j] = (xc0, xc1)
                cp = CPY[j]
                csrc = x0t[0][0:64, r0:r0 + 8, :]
                if cp == "g":
                    nc.gpsimd.tensor_copy(xc0[0:64, :, :], csrc)
                elif cp == "a":
                    nc.scalar.copy(xc0[0:64, :, :], csrc)
                else:
                    nc.vector.tensor_copy(xc0[0:64, :, :], csrc)
                if B1A[j] == "z":
                    grp_pe(TAPS_B1[:Z_NPE], x0t[0][:], d1t[:], bb1t[:],
                           xc0[:], 64, r0, 8, 0, psbrpool, "br1")

            def emit_branch_late_z(j, xc0):
                """dy>0 br1 taps: DVE products + Pool tensor_add onto xc0."""
                r0 = j * 8
                for t, (dy, dx) in enumerate(TAPS_B1):
                    if t < Z_NPE:
                        continue
                    c = _clip(dy, dx, r0, 8)
                    if c is None:
                        continue
                    rlo, rhi, clo, chi = c
                    tmp = tmpool.tile([128, 8, 128], f16, tag="ztmp",
                                      name="ztmp", bufs=6)
                    nc.vector.tensor_scalar(
                        out=tmp[64:128, rlo:rhi, clo:chi],
                        in0=x0t[0][64:128, r0 + dy + rlo: r0 + dy + rhi,
                                   dx + clo: dx + chi],
                        scalar1=k1t[64:128, t:t + 1], scalar2=None,
                        op0=mult)
                    nc.gpsimd.tensor_add(
                        xc0[64:128, rlo:rhi, clo:chi],
                        xc0[64:128, rlo:rhi, clo:chi],
                        tmp[64:128, rlo:rhi, clo:chi])

            def emit_branch_pw(j):
                r0 = j * 8
                xc0, xc1 = xcg.pop(j)
                # branch2 (5x5 dil3) on x0 blk1 -> xc1 (all 128 ch)
                emit_group(B2A[j], TAPS_B2, x0t[1][:], d2t[:], k2t, bb2t[:],
                           xc1[:], 0, r0, 8, 0, psbrpool, "br2")
                # branch1 (3x3 dil3) on x0 blk0 ch64..127 -> xc0[64:]
                if B1A[j] == "z":
                    emit_branch_late_z(j, xc0)
                else:
                    emit_group(B1A[j], TAPS_B1, x0t[0][:], d1t[:], k1t,
                               bb1t[:], xc0[:], 64, r0, 8, 0, psbrpool,
                               "br1")
                # pointwise GEMM per 4-row window
                for sub in range(2):
                    rr = r0 + sub * 4
                    ev = PWE[j * 2 + sub]
                    for ob in range(2):
                        pw = pspwpool.tile([128, 4, 128], f32, tag="pw", name="pw")
                        nc.tensor.matmul(
                            pw[:], lhsT=wpwt[0][:, ob * 128:(ob + 1) * 128],
                            rhs=xc0[:, sub * 4:sub * 4 + 4, :],
                            start=True, stop=False)
                        nc.tensor.matmul(
                            pw[:], lhsT=wpwt[1][:, ob * 128:(ob + 1) * 128],
                            rhs=xc1[:, sub * 4:sub * 4 + 4, :],
                            start=False, stop=True)
                        ys = yspool.tile([128, 4, 128], f16, tag=f"ys{ob}",
                                         name=f"ys{ob}")
                        if ev == "a":
                            nc.scalar.activation(
                                out=ys[:], in_=pw[:], func=IDENT,
                                bias=bpwt[ob][:], scale=1.0)
                        else:
                            nc.vector.tensor_scalar(
                                out=ys[:], in0=pw[:], scalar1=1.0,
                                scalar2=bpwt[ob][:], op0=mult, op1=add)
                        nc.sync.dma_start(
                            out=y_ap[ob * 128:(ob + 1) * 128, rr:rr + 4, :],
                            in_=ys[:])

            # ---------- main pipeline ----------
            def emit_s1(j, hgt):
                for blk in (1, 0):
                    kind = S1A[j * 2 + blk]
                    emit_group(kind, TAPS_S1, xt[blk][:], d0t[blk][:],
                               k0t[blk], be0t[:] if blk == 0 else be1t[:],
                               x0t[blk][:], 0, j * 8, hgt, j * 8, ps1pool,
                               "s1")

            if S1R4:
                # warmup pairs 0,1 (PE), then 16-row double-pair chains
                for j in (0, 1):
                    emit_s1(j, 8)
                    emit_branch_early(j)
                for d in range(1, 8):
                    emit_s1(2 * d, 16)
                    emit_branch_early(2 * d)
                    emit_branch_early(2 * d + 1)
                    emit_branch_pw(2 * d - 2)
                    emit_branch_pw(2 * d - 1)
                emit_branch_pw(14)
                emit_branch_pw(15)
            else:
                for j in range(16):
                    emit_s1(j, 8)
                    emit_branch_early(j)
                    if j >= LAG:
                        emit_branch_pw(j - LAG)
                for j in range(16 - LAG, 16):
                    emit_branch_pw(j)
    return nc


def _prep_aux(w0, b0, w1, b1, w2, b2, w_pw, b_pw, f16):

    d0 = np.zeros((2, 128, 9 * 128), dtype=f16)
    k0sv = np.zeros((2, 128, 9), np.float32)
    for blk in range(2):
        for t, (dy, dx) in enumerate(TAPS_S1):
            vals = w0[blk * 128:(blk + 1) * 128, 0, dy + 1, dx + 1]
            np.fill_diagonal(d0[blk, :, t * 128:(t + 1) * 128],
                             vals.astype(f16))
            k0sv[blk, :, t] = vals
    d1 = np.zeros((128, 9 * 128), dtype=f16)
    k1sv = np.zeros((128, 9), np.float32)
    for t, (dy, dx) in enumerate(TAPS_B1):
        vals = np.zeros(128, np.float32)
        vals[64:128] = w1[:, 0, dy // 3 + 1, dx // 3 + 1]
        np.fill_diagonal(d1[:, t * 128:(t + 1) * 128], vals.astype(f16))
        k1sv[:, t] = vals
    d2 = np.zeros((128, 25 * 128), dtype=f16)
    k2sv = np.zeros((128, 25), np.float32)
    for t, (dy, dx) in enumerate(TAPS_B2):
        v = w2[:, 0, dy // 3 + 2, dx // 3 + 2]
        vals = np.concatenate([v, v])
        np.fill_diagonal(d2[:, t * 128:(t + 1) * 128], vals.astype(f16))
        k2sv[:, t] = vals
    wpw = np.zeros((2, 128, 256), dtype=f16)
    for k in range(2):
        wpw[k] = np.ascontiguousarray(
            w_pw[:, k * 128:(k + 1) * 128].T).astype(f16)

    be0 = b0[0:128].copy()
    be1 = b0[128:256].copy()
    bb1 = np.concatenate([np.zeros(64, np.float32), b1])
    bb2 = np.concatenate([b2, b2])
    return dict(
        d0=d0, d1=d1, d2=d2, wpw=wpw, k0s=k0sv, k1s=k1sv, k2s=k2sv,
        be0=be0.reshape(128, 1).astype(np.float32),
        be1=be1.reshape(128, 1).astype(np.float32),
        bb1=bb1.reshape(128, 1).astype(np.float32),
        bb2=bb2.reshape(128, 1).astype(np.float32),
        bpw=b_pw.reshape(2, 128, 1).astype(np.float32),
    )


def kernel(x, w0, b0, w1, b1, w2, b2, w_pw, b_pw):
    import concourse.mybir as mybir
    from concourse.bass_utils import run_bass_kernel_spmd

    f16 = mybir.dt.np(mybir.dt.float16)

    if "nc" not in _CACHE:
        nc = _build_nc()
        _split_excess_waits(nc, mybir)
        _CACHE["nc"] = nc
    nc = _CACHE["nc"]

    x = np.asarray(x, np.float32)
    aux = _prep_aux(
        np.asarray(w0, np.float32), np.asarray(b0, np.float32),
        np.asarray(w1, np.float32), np.asarray(b1, np.float32),
        np.asarray(w2, np.float32), np.asarray(b2, np.float32),
        np.asarray(w_pw, np.float32), np.asarray(b_pw, np.float32),
        f16,
    )
    in_maps = [
        {"xb": np.ascontiguousarray(x[i].reshape(2, 128, H, W)).astype(f16),
         **aux}
        for i in range(B)
    ]
    res = run_bass_kernel_spmd(nc, in_maps, core_ids=list(range(B)))
    _CACHE["last_result"] = res
    return np.stack([res.results[i]["y"] for i in range(B)]).astype(np.float32)


# revision 5
# speedup vs baseline: 1.0608x; 1.0266x over previous
"""Trainium2 Bass kernel for nn_MPDWConv (B=8, E=256, H=W=128), v2.

Sharding: data-parallel over batch — each of the 8 NeuronCores processes one
full image.

Per-core design (channel-major [c, h, w], fp16 datapath, f32 PSUM):
  * No guard padding anywhere: every conv tap is emitted on the clipped
    row/col range it is valid for (center tap first covers the full window,
    so PSUM accumulation / in-place chains stay exact at the borders).
  * x arrives via banded strided HBM->SBUF DMAs straight into [128,128,128]
    tiles; y leaves straight from PSUM (f32) after the pointwise GEMM.
  * b_pw is folded upstream: delta = w_pw^-1 @ b_pw is added to the biases
    of the xc-producing stages (chunk0 via S1-evac bias on ch0..63, branches
    via their evac/chain biases), so the PW needs no bias at all.
  * Depthwise work (stage-1 3x3, branch 3x3 dil3, branch 5x5 dil3) is
    distributed across all four engines, per window-pair (8 rows):
      'p': diagonal matmuls on PE accumulating in PSUM + Act-evac w/ bias
      'v': DVE tensor_scalar (4x mode) products + tensor_tensor (2x) adds
      'c': Act activation products (scale/bias APs) + DVE tensor_tensor adds
      'g': DVE first tap (w/ bias), then GpSimd scalar_tensor_tensor chain
  * Pointwise conv: dense fp16 GEMM on PE (2 K-chunks x 2 out-blocks per
    4-row window), PSUM -> HBM DMA issued by the sync engine.
"""

import os as _os

import numpy as np

B, E, H, W = 8, 256, 128, 128

# Tap offset tables, center (0,0) first so the first op of every scheme
# covers the full window.
def _mk_taps(offs):
    taps = [(dy, dx) for dy in offs for dx in offs]
    taps.remove((0, 0))
    taps.sort(key=lambda t: (t[0] > 0, t))
    return [(0, 0)] + taps

TAPS_S1 = _mk_taps((-1, 0, 1))            # stage-1 3x3, dilation 1
TAPS_B1 = _mk_taps((-3, 0, 3))            # branch 3x3, dilation 3
TAPS_B2 = _mk_taps((-6, -3, 0, 3, 6))     # branch 5x5, dilation 3

# Engine assignment (tunable via env for sweeps):
#  S1A: 32 slots, window-pair j in 0..15 x blk in 0..1 (index j*2+blk)
#  B1A/B2A: 16 slots each (per window-pair)
S1A = _os.environ.get("S1A", "vpvpvp" + "cv" * 13)
B1A = _os.environ.get("B1A", "z" * 14 + "cc")
# chunk0 passthrough copy engine per pair: g=GpSimd, v=DVE, a=Act
CPY = _os.environ.get("CPY", "g" * 16)
# of the br1 dy<=0 taps, how many go to PE in scheme z
Z_NPE = int(_os.environ.get("Z_NPE", "6"))
# of the 8 non-center adds in scheme c, how many go to Pool
C_NPOOL = int(_os.environ.get("C_NPOOL", "0"))
B2A = _os.environ.get("B2A", "p" * 14 + "yy")
# PW PSUM->SBUF evac engine per 4-row window (32 slots): a=Act, v=DVE, g=GpSimd
PWE = _os.environ.get("PWE", "a" * 28 + "avav")
LAG = int(_os.environ.get("LAG", "1"))
S1R4 = int(_os.environ.get("S1R4", "0"))
U_NV = int(_os.environ.get("U_NV", "2"))
M_NA = int(_os.environ.get("M_NA", "2"))

_CACHE = {}


def _split_excess_waits(nc, mybir):
    """Walrus legalization: TRN2 instructions encode at most ONE sync wait
    (two for EventSemaphore). Tile attaches multi-wait sync_info; move the
    excess onto same-engine NoOp prefixes."""
    n_created = 0
    for fn in nc.m.functions:
        for blk in fn.blocks:
            insts = list(blk.instructions)
            out = []
            changed = False
            for inst in insts:
                si = getattr(inst, "sync_info", None)
                cap = 2 if isinstance(inst, mybir.InstEventSemaphore) else 1
                if si is not None and si.on_wait is not None \
                        and len(si.on_wait) > cap:
                    waits = list(si.on_wait)
                    extra, keep = waits[:-cap], waits[-cap:]
                    for w in extra:
                        n_created += 1
                        nop = mybir.InstNoOp(
                            name=f"I-waitsplit-{n_created}",
                            engine=inst.engine)
                        nop.sync_info = mybir.SyncInfo(
                            on_wait=[w], on_update=[])
                        out.append(nop)
                    inst.sync_info = mybir.SyncInfo(
                        on_wait=keep, on_update=list(si.on_update))
                    changed = True
                out.append(inst)
            if changed:
                blk.instructions = out
    return n_created


def _clip(dy, dx, r0, hgt):
    """Valid local (row, col) ranges of a window [r0, r0+hgt) x [0, 128)
    for a tap reading (row+dy, col+dx). Returns None if empty."""
    rlo = max(0, -r0 - dy)
    rhi = min(hgt, 128 - r0 - dy)
    clo = max(0, -dx)
    chi = min(128, 128 - dx)
    if rhi <= rlo or chi <= clo:
        return None
    return rlo, rhi, clo, chi


def _build_nc():
    import concourse.bass as bass
    import concourse.mybir as mybir
    from concourse import tile

    f16 = mybir.dt.float16
    f32 = mybir.dt.float32
    mult, add = mybir.AluOpType.mult, mybir.AluOpType.add
    IDENT = mybir.ActivationFunctionType.Identity

    nc = bass.Bass(trn_type="TRN2")

    # ---- DRAM parameters ----
    xb = nc.dram_tensor("xb", [2, 128, H, W], f16, kind="ExternalInput")
    d0 = nc.dram_tensor("d0", [2, 128, 9 * 128], f16, kind="ExternalInput")
    d1 = nc.dram_tensor("d1", [128, 9 * 128], f16, kind="ExternalInput")
    d2 = nc.dram_tensor("d2", [128, 25 * 128], f16, kind="ExternalInput")
    wpw = nc.dram_tensor("wpw", [2, 128, 256], f16, kind="ExternalInput")
    k0s = nc.dram_tensor("k0s", [2, 128, 9], f32, kind="ExternalInput")
    k1s = nc.dram_tensor("k1s", [128, 9], f32, kind="ExternalInput")
    k2s = nc.dram_tensor("k2s", [128, 25], f32, kind="ExternalInput")
    be0 = nc.dram_tensor("be0", [128, 1], f32, kind="ExternalInput")
    be1 = nc.dram_tensor("be1", [128, 1], f32, kind="ExternalInput")
    bb1 = nc.dram_tensor("bb1", [128, 1], f32, kind="ExternalInput")
    bb2 = nc.dram_tensor("bb2", [128, 1], f32, kind="ExternalInput")
    bpw = nc.dram_tensor("bpw", [2, 128, 1], f32, kind="ExternalInput")
    y = nc.dram_tensor("y", [E, H, W], f16, kind="ExternalOutput")

    xb_ap, y_ap = xb.ap(), y.ap()

    with tile.TileContext(nc) as tc:
        with (
            tc.tile_pool(name="const", bufs=1) as cpool,
            tc.tile_pool(name="xin", bufs=1) as xpool,
            tc.tile_pool(name="x0", bufs=1) as x0pool,
            tc.tile_pool(name="xcg", bufs=4) as xcpool,
            tc.tile_pool(name="tmps", bufs=3) as tmpool,
            tc.tile_pool(name="ys", bufs=3) as yspool,
            tc.tile_pool(name="ps_s1", bufs=1, space="PSUM") as ps1pool,
            tc.tile_pool(name="ps_br", bufs=2, space="PSUM") as psbrpool,
            tc.tile_pool(name="ps_pw", bufs=3, space="PSUM") as pspwpool,
        ):
            # ---- constants into SBUF (issued on sync engine / HWDGE) ----
            def cdma(shape, dt_, tag, src_ap):
                t = cpool.tile(shape, dt_, tag=tag, name=tag)
                nc.sync.dma_start(out=t[:], in_=src_ap)
                return t

            d0t = [cdma([128, 9 * 128], f16, f"d0_{b}", d0.ap()[b])
                   for b in range(2)]
            d1t = cdma([128, 9 * 128], f16, "d1", d1.ap())
            d2t = cdma([128, 25 * 128], f16, "d2", d2.ap())
            wpwt = [cdma([128, 256], f16, f"wpw_{k}", wpw.ap()[k])
                    for k in range(2)]
            k0t = [cdma([128, 9], f32, f"k0_{b}", k0s.ap()[b])
                   for b in range(2)]
            k1t = cdma([128, 9], f32, "k1", k1s.ap())
            k2t = cdma([128, 25], f32, "k2", k2s.ap())
            be0t = cdma([128, 1], f32, "be0", be0.ap())
            be1t = cdma([128, 1], f32, "be1", be1.ap())
            bb1t = cdma([128, 1], f32, "bb1", bb1.ap())
            bb2t = cdma([128, 1], f32, "bb2", bb2.ap())
            bpwt = [cdma([128, 1], f32, f"bpw_{ob}", bpw.ap()[ob])
                    for ob in range(2)]

            # Pre-touch scalar tiles on their consumer engines so steady-state
            # ops don't each carry an extra DMA-lane sync wait.
            scrV = cpool.tile([128, 1], f32, tag="scrV")
            scrA = cpool.tile([128, 1], f32, tag="scrA")
            scrG = cpool.tile([128, 1], f32, tag="scrG")
            for t in (k0t[0], k0t[1], k1t, k2t, be0t, be1t, bb1t, bb2t,
                      bpwt[0], bpwt[1]):
                nc.vector.tensor_copy(scrV[:], t[:, 0:1])
            for t in (k0t[0], k0t[1], k1t, k2t, be0t, be1t, bb1t, bb2t,
                      bpwt[0], bpwt[1]):
                nc.scalar.copy(scrA[:], t[:, 0:1])
            for t in (k0t[0], k0t[1], k1t, k2t):
                nc.gpsimd.tensor_copy(scrG[:], t[:, 0:1])

            # ---- input tiles: banded strided DMA, no padding ----
            xt = [xpool.tile([128, 128, 128], f16, tag=f"x{b}",
                              name=f"x{b}") for b in range(2)]
            bands = [(0, 8), (8, 8)] + [(r, 16) for r in range(16, 128, 16)]
            for r, h in bands:
                for blk in (1, 0):
                    nc.sync.dma_start(
                        out=xt[blk][:, r:r + h, :],
                        in_=xb_ap[blk, :, r:r + h, :])

            # ---- x0 tiles ----
            x0t = [x0pool.tile([128, 128, 128], f16, tag=f"x0_{b}",
                                name=f"x0_{b}") for b in range(2)]

            # ---------- scheme emitters ----------
            # Each emits one depthwise group over window rows [r0, r0+hgt)
            # writing `out_ap(rlo, rhi, clo, chi)` slices of the destination
            # (partition range already applied by caller via tiles/slices).

            def grp_pe(taps, src, dmat, bias_ap, dst, plo, r0, hgt, dr0,
                       pspool, tag):
                """PE diag matmuls into PSUM (4-row sub-windows) + Act evac.
                src rows are absolute; dst rows start at dr0."""
                for sub in range(0, hgt, 4):
                    rr = r0 + sub
                    ems = []
                    for t, (dy, dx) in enumerate(taps):
                        c = _clip(dy, dx, rr, 4)
                        if c is None:
                            continue
                        ems.append((t, dy, dx, c))
                    ps = pspool.tile([128, 4, 128], f32, tag=tag, name=tag)
                    n = len(ems)
                    for i, (t, dy, dx, (rlo, rhi, clo, chi)) in enumerate(ems):
                        nc.tensor.matmul(
                            ps[:, rlo:rhi, clo:chi],
                            lhsT=dmat[:, t * 128:(t + 1) * 128],
                            rhs=src[:, rr + dy + rlo: rr + dy + rhi,
                                    dx + clo: dx + chi],
                            start=(i == 0), stop=(i == n - 1),
                            skip_group_check=True,
                        )
                    nc.scalar.activation(
                        out=dst[plo:128, dr0 + sub: dr0 + sub + 4, :],
                        in_=ps[plo:128], func=IDENT,
                        bias=bias_ap[plo:128], scale=1.0,
                    )

            def grp_v(taps, src, ktile, bias_ap, dst, plo, r0, hgt, dr0):
                """DVE: ts (4x) center tap w/ bias -> dst, then per tap
                ts product (4x) + tt add (2x), in place on dst."""
                assert taps[0] == (0, 0)
                nc.vector.tensor_scalar(
                    out=dst[plo:128, dr0:dr0 + hgt, :],
                    in0=src[plo:128, r0:r0 + hgt, :],
                    scalar1=ktile[plo:128, 0:1], scalar2=bias_ap[plo:128],
                    op0=mult, op1=add)
                for t, (dy, dx) in enumerate(taps[1:], start=1):
                    c = _clip(dy, dx, r0, hgt)
                    if c is None:
                        continue
                    rlo, rhi, clo, chi = c
                    tmp = tmpool.tile([128, hgt, 128], f16, tag="vtmp", name="vtmp", bufs=6)
                    nc.vector.tensor_scalar(
                        out=tmp[plo:128, rlo:rhi, clo:chi],
                        in0=src[plo:128, r0 + dy + rlo: r0 + dy + rhi,
                                dx + clo: dx + chi],
                        scalar1=ktile[plo:128, t:t + 1], scalar2=None,
                        op0=mult)
                    nc.vector.tensor_add(
                        dst[plo:128, dr0 + rlo: dr0 + rhi, clo:chi],
                        dst[plo:128, dr0 + rlo: dr0 + rhi, clo:chi],
                        tmp[plo:128, rlo:rhi, clo:chi])

            def grp_c(taps, src, ktile, bias_ap, dst, plo, r0, hgt, dr0):
                """Act products (scale AP, bias on center) + DVE tt adds."""
                assert taps[0] == (0, 0)
                nc.scalar.activation(
                    out=dst[plo:128, dr0:dr0 + hgt, :],
                    in_=src[plo:128, r0:r0 + hgt, :],
                    func=IDENT, bias=bias_ap[plo:128],
                    scale=ktile[plo:128, 0:1])
                for t, (dy, dx) in enumerate(taps[1:], start=1):
                    c = _clip(dy, dx, r0, hgt)
                    if c is None:
                        continue
                    rlo, rhi, clo, chi = c
                    tmp = tmpool.tile([128, hgt, 128], f16, tag="ctmp", name="ctmp", bufs=8)
                    nc.scalar.activation(
                        out=tmp[plo:128, rlo:rhi, clo:chi],
                        in_=src[plo:128, r0 + dy + rlo: r0 + dy + rhi,
                                dx + clo: dx + chi],
                        func=IDENT, bias=0.0,
                        scale=ktile[plo:128, t:t + 1])
                    adder = (nc.gpsimd if t > len(taps) - 1 - C_NPOOL
                             else nc.vector)
                    adder.tensor_add(
                        dst[plo:128, dr0 + rlo: dr0 + rhi, clo:chi],
                        dst[plo:128, dr0 + rlo: dr0 + rhi, clo:chi],
                        tmp[plo:128, rlo:rhi, clo:chi])

            def grp_g(taps, src, ktile, bias_ap, dst, plo, r0, hgt, dr0):
                """DVE ts center tap w/ bias -> dst, then GpSimd STT chain."""
                assert taps[0] == (0, 0)
                nc.vector.tensor_scalar(
                    out=dst[plo:128, dr0:dr0 + hgt, :],
                    in0=src[plo:128, r0:r0 + hgt, :],
                    scalar1=ktile[plo:128, 0:1], scalar2=bias_ap[plo:128],
                    op0=mult, op1=add)
                for t, (dy, dx) in enumerate(taps[1:], start=1):
                    c = _clip(dy, dx, r0, hgt)
                    if c is None:
                        continue
                    rlo, rhi, clo, chi = c
                    nc.gpsimd.scalar_tensor_tensor(
                        out=dst[plo:128, dr0 + rlo: dr0 + rhi, clo:chi],
                        in0=src[plo:128, r0 + dy + rlo: r0 + dy + rhi,
                                dx + clo: dx + chi],
                        scalar=ktile[plo:128, t:t + 1],
                        in1=dst[plo:128, dr0 + rlo: dr0 + rhi, clo:chi],
                        op0=mult, op1=add)

            def grp_y(taps, src, dmat, ktile, bias_ap, dst, plo, r0, hgt,
                      dr0, pspool, tag, n_dve=None):
                if n_dve is None:
                    n_pe = 1 + sum(1 for dy, dx in taps[1:] if dy <= 0)
                else:
                    n_pe = len(taps) - n_dve
                grp_pe(taps[:n_pe], src, dmat, bias_ap, dst, plo, r0, hgt,
                       dr0, pspool, tag)
                for t, (dy, dx) in enumerate(taps):
                    if t < n_pe:
                        continue
                    c = _clip(dy, dx, r0, hgt)
                    if c is None:
                        continue
                    rlo, rhi, clo, chi = c
                    tmp = tmpool.tile([128, hgt, 128], f16, tag="vtmp",
                                      name="vtmp", bufs=6)
                    nc.vector.tensor_scalar(
                        out=tmp[plo:128, rlo:rhi, clo:chi],
                        in0=src[plo:128, r0 + dy + rlo: r0 + dy + rhi,
                                dx + clo: dx + chi],
                        scalar1=ktile[plo:128, t:t + 1], scalar2=None,
                        op0=mult)
                    nc.vector.tensor_add(
                        dst[plo:128, dr0 + rlo: dr0 + rhi, clo:chi],
                        dst[plo:128, dr0 + rlo: dr0 + rhi, clo:chi],
                        tmp[plo:128, rlo:rhi, clo:chi])

            def emit_group(kind, taps, src, dmat, ktile, bias_ap, dst, plo,
                           r0, hgt, dr0, pspool, tag):
                if kind == "m":
                    n_pe = len(taps) - M_NA
                    grp_pe(taps[:n_pe], src, dmat, bias_ap, dst, plo, r0,
                           hgt, dr0, pspool, tag)
                    for t, (dy, dx) in enumerate(taps):
                        if t < n_pe:
                            continue
                        c = _clip(dy, dx, r0, hgt)
                        if c is None:
                            continue
                        rlo, rhi, clo, chi = c
                        tmp = tmpool.tile([128, hgt, 128], f16, tag="ctmp",
                                          name="ctmp", bufs=8)
                        nc.scalar.activation(
                            out=tmp[plo:128, rlo:rhi, clo:chi],
                            in_=src[plo:128, r0 + dy + rlo: r0 + dy + rhi,
                                    dx + clo: dx + chi],
                            func=IDENT, bias=0.0,
                            scale=ktile[plo:128, t:t + 1])
                        nc.vector.tensor_add(
                            dst[plo:128, dr0 + rlo: dr0 + rhi, clo:chi],
                            dst[plo:128, dr0 + rlo: dr0 + rhi, clo:chi],
                            tmp[plo:128, rlo:rhi, clo:chi])
                elif kind == "y":
                    grp_y(taps, src, dmat, ktile, bias_ap, dst, plo, r0,
                          hgt, dr0, pspool, tag)
                elif kind == "u":
                    grp_y(taps, src, dmat, ktile, bias_ap, dst, plo, r0,
                          hgt, dr0, pspool, tag, n_dve=U_NV)
                elif kind == "p":
                    grp_pe(taps, src, dmat, bias_ap, dst, plo, r0, hgt, dr0,
                           pspool, tag)
                elif kind == "v":
                    grp_v(taps, src, ktile, bias_ap, dst, plo, r0, hgt, dr0)
                elif kind == "c":
                    grp_c(taps, src, ktile, bias_ap, dst, plo, r0, hgt, dr0)
                elif kind == "g":
                    grp_g(taps, src, ktile, bias_ap, dst, plo, r0, hgt, dr0)
                else:
                    raise ValueError(kind)

            # ---------- branch + pointwise for one window-pair ----------
            xcg = {}

            def emit_branch_early(j):
                """Emitted right after S1(j): chunk0 copy + the br1 taps
                that need no pair-(j+1) rows (dy <= 0)."""
                r0 = j * 8
                xc0 = xcpool.tile([128, 8, 128], f16, tag="xc0g", name="xc0g")
                xc1 = xcpool.tile([128, 8, 128], f16, tag="xc1g", name="xc1g")
                xcg[j] = (xc0, xc1)
                cp = CPY[j]
                csrc = x0t[0][0:64, r0:r0 + 8, :]
                if cp == "g":
                    nc.gpsimd.tensor_copy(xc0[0:64, :, :], csrc)
                elif cp == "a":
                    nc.scalar.copy(xc0[0:64, :, :], csrc)
                else:
                    nc.vector.tensor_copy(xc0[0:64, :, :], csrc)
                if B1A[j] == "z":
                    grp_pe(TAPS_B1[:Z_NPE], x0t[0][:], d1t[:], bb1t[:],
                           xc0[:], 64, r0, 8, 0, psbrpool, "br1")

            def emit_branch_late_z(j, xc0):
                """dy>0 br1 taps: DVE products + Pool tensor_add onto xc0."""
                r0 = j * 8
                for t, (dy, dx) in enumerate(TAPS_B1):
                    if t < Z_NPE:
                        continue
                    c = _clip(dy, dx, r0, 8)
                    if c is None:
                        continue
                    rlo, rhi, clo, chi = c
                    tmp = tmpool.tile([128, 8, 128], f16, tag="ztmp",
                                      name="ztmp", bufs=6)
                    nc.vector.tensor_scalar(
                        out=tmp[64:128, rlo:rhi, clo:chi],
                        in0=x0t[0][64:128, r0 + dy + rlo: r0 + dy + rhi,
                                   dx + clo: dx + chi],
                        scalar1=k1t[64:128, t:t + 1], scalar2=None,
                        op0=mult)
                    nc.gpsimd.tensor_add(
                        xc0[64:128, rlo:rhi, clo:chi],
                        xc0[64:128, rlo:rhi, clo:chi],
                        tmp[64:128, rlo:rhi, clo:chi])

            def emit_branch_pw(j):
                r0 = j * 8
                xc0, xc1 = xcg.pop(j)
                # branch2 (5x5 dil3) on x0 blk1 -> xc1 (all 128 ch)
                emit_group(B2A[j], TAPS_B2, x0t[1][:], d2t[:], k2t, bb2t[:],
                           xc1[:], 0, r0, 8, 0, psbrpool, "br2")
                # branch1 (3x3 dil3) on x0 blk0 ch64..127 -> xc0[64:]
                if B1A[j] == "z":
                    emit_branch_late_z(j, xc0)
                else:
                    emit_group(B1A[j], TAPS_B1, x0t[0][:], d1t[:], k1t,
                               bb1t[:], xc0[:], 64, r0, 8, 0, psbrpool,
                               "br1")
                # pointwise GEMM per 4-row window
                for sub in range(2):
                    rr = r0 + sub * 4
                    ev = PWE[j * 2 + sub]
                    for ob in range(2):
                        pw = pspwpool.tile([128, 4, 128], f32, tag="pw", name="pw")
                        nc.tensor.matmul(
                            pw[:], lhsT=wpwt[0][:, ob * 128:(ob + 1) * 128],
                            rhs=xc0[:, sub * 4:sub * 4 + 4, :],
                            start=True, stop=False)
                        nc.tensor.matmul(
                            pw[:], lhsT=wpwt[1][:, ob * 128:(ob + 1) * 128],
                            rhs=xc1[:, sub * 4:sub * 4 + 4, :],
                            start=False, stop=True)
                        ys = yspool.tile([128, 4, 128], f16, tag=f"ys{ob}",
                                         name=f"ys{ob}")
                        if ev == "a":
                            nc.scalar.activation(
                                out=ys[:], in_=pw[:], func=IDENT,
                                bias=bpwt[ob][:], scale=1.0)
                        else:
                            nc.vector.tensor_scalar(
                                out=ys[:], in0=pw[:], scalar1=1.0,
                                scalar2=bpwt[ob][:], op0=mult, op1=add)
                        nc.sync.dma_start(
                            out=y_ap[ob * 128:(ob + 1) * 128, rr:rr + 4, :],
                            in_=ys[:])

            # ---------- main pipeline ----------
            def emit_s1(j, hgt):
                for blk in (1, 0):
                    kind = S1A[j * 2 + blk]
                    emit_group(kind, TAPS_S1, xt[blk][:], d0t[blk][:],
                               k0t[blk], be0t[:] if blk == 0 else be1t[:],
                               x0t[blk][:], 0, j * 8, hgt, j * 8, ps1pool,
                               "s1")

            if S1R4:
                # warmup pairs 0,1 (PE), then 16-row double-pair chains
                for j in (0, 1):
                    emit_s1(j, 8)
                    emit_branch_early(j)
                for d in range(1, 8):
                    emit_s1(2 * d, 16)
                    emit_branch_early(2 * d)
                    emit_branch_early(2 * d + 1)
                    emit_branch_pw(2 * d - 2)
                    emit_branch_pw(2 * d - 1)
                emit_branch_pw(14)
                emit_branch_pw(15)
            else:
                for j in range(16):
                    emit_s1(j, 8)
                    emit_branch_early(j)
                    if j >= LAG:
                        emit_branch_pw(j - LAG)
                for j in range(16 - LAG, 16):
                    emit_branch_pw(j)
    return nc


def _prep_aux(w0, b0, w1, b1, w2, b2, w_pw, b_pw, f16):

    d0 = np.zeros((2, 128, 9 * 128), dtype=f16)
    k0sv = np.zeros((2, 128, 9), np.float32)
    for blk in range(2):
        for t, (dy, dx) in enumerate(TAPS_S1):
            vals = w0[blk * 128:(blk + 1) * 128, 0, dy + 1, dx + 1]
            np.fill_diagonal(d0[blk, :, t * 128:(t + 1) * 128],
                             vals.astype(f16))
            k0sv[blk, :, t] = vals
    d1 = np.zeros((128, 9 * 128), dtype=f16)
    k1sv = np.zeros((128, 9), np.float32)
    for t, (dy, dx) in enumerate(TAPS_B1):
        vals = np.zeros(128, np.float32)
        vals[64:128] = w1[:, 0, dy // 3 + 1, dx // 3 + 1]
        np.fill_diagonal(d1[:, t * 128:(t + 1) * 128], vals.astype(f16))
        k1sv[:, t] = vals
    d2 = np.zeros((128, 25 * 128), dtype=f16)
    k2sv = np.zeros((128, 25), np.float32)
    for t, (dy, dx) in enumerate(TAPS_B2):
        v = w2[:, 0, dy // 3 + 2, dx // 3 + 2]
        vals = np.concatenate([v, v])
        np.fill_diagonal(d2[:, t * 128:(t + 1) * 128], vals.astype(f16))
        k2sv[:, t] = vals
    wpw = np.zeros((2, 128, 256), dtype=f16)
    for k in range(2):
        wpw[k] = np.ascontiguousarray(
            w_pw[:, k * 128:(k + 1) * 128].T).astype(f16)

    be0 = b0[0:128].copy()
    be1 = b0[128:256].copy()
    bb1 = np.concatenate([np.zeros(64, np.float32), b1])
    bb2 = np.concatenate([b2, b2])
    return dict(
        d0=d0, d1=d1, d2=d2, wpw=wpw, k0s=k0sv, k1s=k1sv, k2s=k2sv,
        be0=be0.reshape(128, 1).astype(np.float32),
        be1=be1.reshape(128, 1).astype(np.float32),
        bb1=bb1.reshape(128, 1).astype(np.float32),
        bb2=bb2.reshape(128, 1).astype(np.float32),
        bpw=b_pw.reshape(2, 128, 1).astype(np.float32),
    )


def kernel(x, w0, b0, w1, b1, w2, b2, w_pw, b_pw):
    import concourse.mybir as mybir
    from concourse.bass_utils import run_bass_kernel_spmd

    f16 = mybir.dt.np(mybir.dt.float16)

    if "nc" not in _CACHE:
        nc = _build_nc()
        _split_excess_waits(nc, mybir)
        _CACHE["nc"] = nc
    nc = _CACHE["nc"]

    x = np.asarray(x, np.float32)
    aux = _prep_aux(
        np.asarray(w0, np.float32), np.asarray(b0, np.float32),
        np.asarray(w1, np.float32), np.asarray(b1, np.float32),
        np.asarray(w2, np.float32), np.asarray(b2, np.float32),
        np.asarray(w_pw, np.float32), np.asarray(b_pw, np.float32),
        f16,
    )
    in_maps = [
        {"xb": np.ascontiguousarray(x[i].reshape(2, 128, H, W)).astype(f16),
         **aux}
        for i in range(B)
    ]
    res = run_bass_kernel_spmd(nc, in_maps, core_ids=list(range(B)))
    _CACHE["last_result"] = res
    return np.stack([res.results[i]["y"] for i in range(B)]).astype(np.float32)
